# revision 9
# baseline (speedup 1.0000x reference)
"""DeepSeek sparse attention (MLA + YaRN RoPE + local/dilated/global mask) on 8 TRN2 cores.

Sharding: (batch, head-group) across 8 cores — core c handles batch c//4, heads
[4*(c%4), 4*(c%4)+4).  Each core computes its projections from the full x (host
pre-transposes x per batch), runs block-sparse attention for its 4 heads, and
produces a row-parallel partial of out @ w_o.  Host sums the 4 partials per batch.

Each core computes the full c_kv latent redundantly: deduplicating it via a
group AllGather was measured to put the chip in a lower power state (~2.0 GHz
PE clock for any collective-enabled NEFF vs 2.4 GHz without), which costs far
more than the duplicated matmuls save.

Layout: "transposed" activations [feature, t] so every matmul keeps the moving
operand in the free dim (N=512/256) at full bf16 rate and no on-chip transposes
are needed anywhere.  Scores are computed as S^T[k, q]; exp tiles are summed on
the Vector engine into one [128, QB] accumulator per (head, block) so the
softmax denominator costs a single ones-matmul; the divide is an elementwise
mul by the reciprocal.
"""

import sys

if "/opt/trn_rl_repo" not in sys.path:
    sys.path.insert(0, "/opt/trn_rl_repo")

import ml_dtypes
import numpy as np

import concourse.bass as bass  # noqa: F401  (bass types used via tile/bacc)
import concourse.mybir as mybir
import concourse.tile as tile
from concourse import bacc, bass_utils

BF16 = ml_dtypes.bfloat16

# ---- problem constants (hardcoded per contract) ----
B, T, D = 2, 2048, 2048
H, DH, DR, DL = 16, 128, 64, 512
WINDOW, STRIDE, GLOB = 512, 64, 128
BASE, MAX_SEQ, ORIG_MAX = 10000.0, 131072, 4096
BETA_FAST, BETA_SLOW = 32.0, 1.0
SCALE = 1.0 / float(np.sqrt(DH))
SCALE_ROPE = 1.0 / float(np.sqrt(DR))
YARN = float(np.float32(0.1 * np.log(MAX_SEQ / ORIG_MAX) + 1.0))
HALF = WINDOW // 2

NCORES = 8
NH = 4            # heads per core
NP = 2            # head-pairs per core (rope tiles stack 2 heads on 128 partitions)
QB = 512          # query block
NB = T // QB      # 4
NBW = (HALF + QB) // 128   # window-strip tiles per block
SL = 512          # t-slice width in projection phase
NS = T // SL      # 4
NT = T // 128     # 16
DLC = DL // 4     # latent dims computed per core (128)
NWARM = 16        # warm-up matmuls: keep PE busy+warm during the input DMA


def _inv_freq():
    base_inv = 1.0 / (BASE ** (np.arange(0, DR, 2, dtype=np.float32) / DR))
    scale = MAX_SEQ / ORIG_MAX
    freqs = np.arange(DR // 2, dtype=np.float32)
    ramp = np.clip((freqs - BETA_SLOW) / (BETA_FAST - BETA_SLOW), 0.0, 1.0)
    return (base_inv * (1 - ramp) + (base_inv / scale) * ramp).astype(np.float32)


def _full_mask():
    pos = np.arange(T)
    qp, kp = pos[:, None], pos[None, :]
    dist = qp - kp
    window = (dist >= -HALF) & (dist <= HALF)
    dil = (kp % STRIDE == 0) | (kp < GLOB)
    return (window | dil) & (kp <= qp)


def _mask_tiles():
    """Per q-block key tiles, with exactly-once ownership masks.

    Tiles: A0 = keys [0, 128) (global), A1 = 32 dilated keys {64j}, B0..B3 =
    the 512-wide sliding window strip.  A0 owns k<128; A1 owns k%64==0 & k>=128;
    B owns the rest.  Each tile classified: 'skip' (all-zero), 'ones', 'mask'.
    """
    full = _full_mask()
    blocks = []
    for qb in range(NB):
        q0 = qb * QB
        k0 = max(0, q0 - HALF)
        qs = slice(q0, q0 + QB)
        blk = []
        m = full[qs, 0:GLOB].T.copy()                        # [128, QB]
        blk.append(dict(kind="A0", i=0, k0=0, keys=np.arange(GLOB), m=m))
        keys = np.arange(32) * STRIDE
        m = full[qs, :][:, keys].T.copy()                    # [32, QB]
        m[keys < GLOB] = False
        blk.append(dict(kind="A1", i=0, k0=0, keys=keys, m=m))
        for i in range(NBW):
            kk = k0 + 128 * i + np.arange(128)
            m = full[qs, :][:, kk].T.copy()
            m[(kk < GLOB) | (kk % STRIDE == 0)] = False
            blk.append(dict(kind="B", i=i, k0=k0, keys=kk, m=m))
        for t_ in blk:
            t_["cls"] = ("skip" if not t_["m"].any()
                         else "ones" if t_["m"].all() else "mask")
            if t_["cls"] == "skip":
                t_["qr"] = (0, QB)
            else:
                cols = np.flatnonzero(t_["m"].any(axis=0))
                a, b = int(cols[0]), int(cols[-1]) + 1
                assert (t_["m"].any(axis=0)[a:b]).all()  # contiguous
                t_["qr"] = (a, b)
        # accumulation groups start with tile 0: it must span all q columns
        assert blk[0]["qr"] == (0, QB)
        blocks.append(blk)
    # exactly-once coverage check against the reference mask
    for qb in range(NB):
        cov = np.zeros((QB, T), dtype=np.int32)
        for t_ in blocks[qb]:
            cov[np.arange(QB)[:, None], t_["keys"][None, :]] += t_["m"].T
        assert (cov == full[qb * QB:(qb + 1) * QB].astype(np.int32)).all()
    return blocks


_MASK_TILES = _mask_tiles()


def _build_program():
    nc = bacc.Bacc("TRN2", target_bir_lowering=False, debug=False,
                   enable_asserts=False, num_devices=NCORES)
    bf, f32 = mybir.dt.bfloat16, mybir.dt.float32

    xT = nc.dram_tensor("xT", [D, T], bf, kind="ExternalInput").ap()
    w_q = nc.dram_tensor("w_q", [D, NH * DH], bf, kind="ExternalInput").ap()
    w_dkv = nc.dram_tensor("w_dkv", [D, DL], bf, kind="ExternalInput").ap()
    w_uk = nc.dram_tensor("w_uk", [DL, NH * DH], bf, kind="ExternalInput").ap()
    w_uv = nc.dram_tensor("w_uv", [DL, NH * DH], bf, kind="ExternalInput").ap()
    w_qp = nc.dram_tensor("w_qp", [D, NH * DR], bf, kind="ExternalInput").ap()
    w_kp = nc.dram_tensor("w_kp", [D, NH * DR], bf, kind="ExternalInput").ap()
    w_o = nc.dram_tensor("w_o", [NH * DH, D], bf, kind="ExternalInput").ap()
    cosT = nc.dram_tensor("cosT", [128, T], f32, kind="ExternalInput").ap()
    ssgnT = nc.dram_tensor("ssgnT", [128, T], f32, kind="ExternalInput").ap()
    mA0 = nc.dram_tensor("mA0", [GLOB, QB], bf, kind="ExternalInput").ap()
    mA1 = nc.dram_tensor("mA1", [NB, 32, QB], bf, kind="ExternalInput").ap()
    mB = nc.dram_tensor("mB", [NB, NBW, 128, QB], bf, kind="ExternalInput").ap()
    out = nc.dram_tensor("out", [T, D], bf, kind="ExternalOutput").ap()

    EXP = mybir.ActivationFunctionType.Exp

    with tile.TileContext(nc) as tc:
        with tc.tile_pool(name="acts", bufs=1) as acts, \
             tc.tile_pool(name="consts", bufs=1) as consts:
            q_cT = [acts.tile([128, T], bf, tag=f"q_cT{h}", name=f"q_cT{h}") for h in range(NH)]
            k_cT = [acts.tile([128, T], bf, tag=f"k_cT{h}", name=f"k_cT{h}") for h in range(NH)]
            # per-head q_rope tiles: head h's 64 rope dims live in rows
            # po..po+64 (po = (h%2)*64); the other 64 rows are ZERO so the
            # score matmul can run full-row (128-contraction) against the
            # pair-stacked k_rT stationary -- zeros kill the partner head's
            # contribution.  Full-row MMs keep LDWEIGHTS pull-ahead alive
            # (row_grp MMs were measured to serialize with ~250ns bubbles).
            q_pad = [acts.tile([128, T], bf, tag=f"q_pad{h}", name=f"q_pad{h}") for h in range(NH)]
            k_rT = [acts.tile([128, T], bf, tag=f"k_rT{p}", name=f"k_rT{p}") for p in range(NP)]
            v_t = [acts.tile([128, NH * DH], bf, tag=f"v{t_}", name=f"v{t_}") for t_ in range(NT)]
            v_A = acts.tile([32, NH * DH], bf, tag="v_A", name="v_A")
            for h in range(NH):
                po = (h % 2) * 64
                nc.vector.memset(q_pad[h][64 - po:128 - po, :], 0.0)
            ones = consts.tile([128, 128], bf, tag="ones", name="ones")
            nc.vector.memset(ones, 1.0)
            junk = consts.tile([128, 128], bf, tag="junk", name="junk")
            nc.vector.memset(junk, 0.001)
            # load the Exp LUT into ACT early: the first real exp otherwise
            # pays a 1.3us table load right when P2's PSUM rotation is tight
            warm = consts.tile([1, 2], f32, tag="warm", name="warm")
            nc.vector.memset(warm, 0.0)
            nc.scalar.activation(out=warm, in_=warm, func=EXP)

            # ---------------- P1: projections (t-slice streamed) --------------
            with tc.tile_pool(name="wp1", bufs=1) as wp1, \
                 tc.tile_pool(name="xch", bufs=2) as xch, \
                 tc.tile_pool(name="ckvp", bufs=1) as ckvp, \
                 tc.tile_pool(name="rope_t", bufs=3) as rope_t, \
                 tc.tile_pool(name="p1ps", bufs=7, space="PSUM") as p1ps, \
                 tc.tile_pool(name="vAps", bufs=1, space="PSUM") as vAps:
                # full c_kv^T latent (4 chunks of 128 dims x T), P1-scoped
                ckv = [ckvp.tile([128, T], bf, tag=f"ckv{g}", name=f"ckv{g}")
                       for g in range(4)]
                # warm-up matmuls: no data deps, so they issue from t=0 and
                # keep the PE busy (and the HAM clock un-throttled) while the
                # first x/weight DMAs land
                wu = p1ps.tile([128, SL], f32, tag="proj", name="wu")
                for _ in range(NWARM):
                    nc.tensor.matmul(wu[:, 0:128], ones, junk,
                                     start=True, stop=True)

                # per-d-chunk tiles for w_q and the first x slice: Tile's DMA
                # dependency is tile-granular, so separate tiles let the d=0
                # matmuls start after ~1/16 of the load instead of all of it
                wq_d = [wp1.tile([128, NH * DH], bf, tag=f"wq{d}", name=f"wq{d}")
                        for d in range(16)]
                x0_d = [xch.tile([128, SL], bf, tag=f"x{d}", name=f"x{d}")
                        for d in range(16)]
                wdkv_sb = wp1.tile([128, 16 * DL], bf, tag="wdkv", name="wdkv")
                wqp_sb = wp1.tile([128, 16 * NH * DR], bf, tag="wqp", name="wqp")
                wkp_sb = wp1.tile([128, 16 * NH * DR], bf, tag="wkp", name="wkp")
                wuk_sb = wp1.tile([128, 4 * NH * DH], bf, tag="wuk", name="wuk")
                wuv_sb = wp1.tile([128, 4 * NH * DH], bf, tag="wuv", name="wuv")
                cos_sb = wp1.tile([128, T], f32, tag="cos", name="cos")
                ssg_sb = wp1.tile([128, T], f32, tag="ssg", name="ssg")

                def _wslice(big, cols, d, c0, c1):
                    return big[:, d * cols + c0:d * cols + c1]

                def _load_w(dst, src, cols):
                    nc.sync.dma_start(
                        out=dst.rearrange("p (n m) -> p n m", m=cols),
                        in_=src.rearrange("(n p) m -> p n m", p=128))

                # ordered so slice-0 compute can start ASAP: interleave the
                # per-d x / w_q chunks first (consumed in d order), then the
                # weights in the order the schedule needs them
                wq_r = w_q.rearrange("(n p) m -> p n m", p=128)
                xT_r = xT.rearrange("(n p) m -> p n m", p=128)
                for d in range(16):
                    nc.sync.dma_start(out=x0_d[d], in_=xT_r[:, d, 0:SL])
                    nc.sync.dma_start(out=wq_d[d], in_=wq_r[:, d])
                _load_w(wdkv_sb, w_dkv, DL)
                _load_w(wqp_sb, w_qp, NH * DR)
                nc.sync.dma_start(out=cos_sb, in_=cosT)
                nc.sync.dma_start(out=ssg_sb, in_=ssgnT)
                _load_w(wkp_sb, w_kp, NH * DR)
                _load_w(wuk_sb, w_uk, NH * DH)
                _load_w(wuv_sb, w_uv, NH * DH)

                COPY = mybir.ActivationFunctionType.Copy
                xcur = x0_d
                for s in range(NS):
                    t0 = s * SL
                    tsl = slice(t0, t0 + SL)
                    xnxt = None
                    if s + 1 < NS:  # prefetch next slice (per-d chunk tiles)
                        xnxt = [xch.tile([128, SL], bf, tag=f"x{d}",
                                         name=f"x{d}") for d in range(16)]
                        for d in range(16):
                            nc.sync.dma_start(
                                out=xnxt[d],
                                in_=xT_r[:, d, (s + 1) * SL:(s + 2) * SL])
                    xt = xcur
                    # content q projections, d-outer so the first matmuls only
                    # need the first DMA chunk of x/w_q (fast start on slice 0)
                    qps = [p1ps.tile([128, SL], f32, tag="proj", name="proj")
                           for _ in range(NH)]
                    for d in range(16):
                        for h in range(NH):
                            nc.tensor.matmul(
                                qps[h], wq_d[d][:, h * DH:(h + 1) * DH],
                                xt[d], start=(d == 0), stop=(d == 15))
                    for h in range(NH):
                        nc.vector.tensor_copy(out=q_cT[h][:, tsl], in_=qps[h])
                    # full c_kv latent for this slice, into the persistent
                    # ckv tiles (4 chunks of 128 latent dims)
                    for g in range(4):
                        ps = p1ps.tile([128, SL], f32, tag="proj", name="proj")
                        for d in range(16):
                            nc.tensor.matmul(
                                ps, _wslice(wdkv_sb, DL, d, g * 128, (g + 1) * 128),
                                xt[d], start=(d == 0), stop=(d == 15))
                        nc.vector.tensor_copy(out=ckv[g][:, tsl], in_=ps)
                    # rope projections + rotation (pair-stacked: 2 heads / tile)
                    for w_sb, isq in ((wqp_sb, True), (wkp_sb, False)):
                        for p in range(NP):
                            ps = p1ps.tile([128, SL], f32, tag="proj", name="proj")
                            for d in range(16):
                                nc.tensor.matmul(
                                    ps, _wslice(w_sb, NH * DR, d, p * 128, (p + 1) * 128),
                                    xt[d], start=(d == 0), stop=(d == 15))
                            m1 = rope_t.tile([128, SL], bf, tag="m1", name="m1")
                            nc.vector.tensor_mul(m1, ps, cos_sb[:, tsl])
                            m2 = rope_t.tile([128, SL], bf, tag="m2", name="m2")
                            for a in (0, 32, 64, 96):
                                sw = a ^ 32
                                nc.vector.tensor_mul(m2[a:a + 32, :],
                                                     ps[sw:sw + 32, :],
                                                     ssg_sb[a:a + 32, tsl])
                            if isq:
                                # split into the per-head zero-padded tiles
                                nc.vector.tensor_add(
                                    q_pad[2 * p][0:64, tsl],
                                    m1[0:64, :], m2[0:64, :])
                                nc.vector.tensor_add(
                                    q_pad[2 * p + 1][64:128, tsl],
                                    m1[64:128, :], m2[64:128, :])
                            else:
                                nc.vector.tensor_add(k_rT[p][:, tsl], m1, m2)
                    xcur = xnxt

                # k content + v from the latent; PSUM->SBUF staging on the
                # Scalar engine here -- Vector must be drained by the time the
                # first attention block's exp/acc chain starts, or the early
                # score matmuls stall on PSUM recycling
                for s in range(NS):
                    t0 = s * SL
                    tsl = slice(t0, t0 + SL)
                    for h in range(NH):
                        ps = p1ps.tile([128, SL], f32, tag="proj", name="proj")
                        for g in range(4):
                            nc.tensor.matmul(
                                ps, _wslice(wuk_sb, NH * DH, g, h * DH, (h + 1) * DH),
                                ckv[g][:, tsl], start=(g == 0), stop=(g == 3))
                        nc.scalar.activation(out=k_cT[h][:, tsl], in_=ps,
                                             func=COPY)
                    # v in [t, dh] layout
                    for u in range(4):
                        usl = slice(t0 + u * 128, t0 + (u + 1) * 128)
                        ps = p1ps.tile([128, NH * DH], f32, tag="proj", name="proj")
                        for g in range(4):
                            nc.tensor.matmul(
                                ps, ckv[g][:, usl],
                                wuv_sb[:, g * NH * DH:(g + 1) * NH * DH],
                                start=(g == 0), stop=(g == 3))
                        nc.vector.tensor_copy(out=v_t[s * 4 + u], in_=ps)
                # dilated-key V rows (keys 64j), one pass over full T
                psA = vAps.tile([32, NH * DH], f32, tag="vA", name="vA")
                for g in range(4):
                    nc.tensor.matmul(psA, ckv[g][:, 0:T:STRIDE],
                                     wuv_sb[:, g * NH * DH:(g + 1) * NH * DH],
                                     start=(g == 0), stop=(g == 3))
                nc.vector.tensor_copy(out=v_A, in_=psA)

            # ---------------- P2: block-sparse attention ----------------------
            with tc.tile_pool(name="wo", bufs=1) as wo, \
                 tc.tile_pool(name="attp", bufs=1) as attp:
              att = [attp.tile([128, T], bf, tag=f"att{h}", name=f"att{h}")
                     for h in range(NH)]
              w_o_sb = wo.tile([128, NH * D], bf, tag="wo", name="wo")
              nc.sync.dma_start(
                  out=w_o_sb.rearrange("p (n m) -> p n m", m=D),
                  in_=w_o.rearrange("(n p) m -> p n m", p=128))

              with tc.tile_pool(name="mk", bufs=2) as mk, \
                   tc.tile_pool(name="exp", bufs=48) as expp, \
                   tc.tile_pool(name="dacc", bufs=12) as dacc, \
                   tc.tile_pool(name="p2t", bufs=3) as p2t, \
                   tc.tile_pool(name="p3s", bufs=4) as p3s, \
                   tc.tile_pool(name="scps", bufs=4, space="PSUM") as scps, \
                   tc.tile_pool(name="daops", bufs=2, space="PSUM") as daops, \
                   tc.tile_pool(name="p3ps", bufs=2, space="PSUM") as p3ps:
                rcs = {}
                accs = {}

                def p2_chunks(qb, exps):
                    """pass 2 of block qb as deferred emitters (2 per head).

                    Enqueued as dn0, dn1, ao0, dn2, ao1, dn3, ao2, ao3 so a
                    head's reciprocal (Vector) runs under the next head's dn /
                    interleaved score matmuls instead of stalling the in-order
                    PE queue.
                    """
                    q0 = qb * QB
                    qsl = slice(q0, q0 + QB)
                    blk = [t_ for t_ in _MASK_TILES[qb] if t_["cls"] != "skip"]
                    last = len(blk) - 1

                    def dn_chunk(h):
                        dn = scps.tile([128, QB], f32, tag="sc", name="dn")
                        nc.tensor.matmul(dn, ones, accs.pop((qb, h)),
                                         start=True, stop=True)
                        rc = p2t.tile([128, QB], f32, tag="rc", name="rc")
                        nc.vector.reciprocal_approx_fast(out=rc, in_=dn)
                        rcs[(qb, h)] = rc

                    def ao_chunk(h):
                        hs = slice(h * DH, (h + 1) * DH)
                        ao = daops.tile([128, QB], f32, tag="dao", name="ao")
                        for j, t_ in enumerate(blk):
                            kind, i, k0 = t_["kind"], t_["i"], t_["k0"]
                            rows, ex = exps[(h, kind, i)]
                            if kind == "A0":
                                lv = v_t[0][:, hs]
                            elif kind == "A1":
                                lv = v_A[:, hs]
                            else:
                                lv = v_t[k0 // 128 + i][:, hs]
                            a, b = t_["qr"]
                            nc.tensor.matmul(ao[:, a:b], lv[0:rows, :],
                                             ex[0:rows, 0:b - a],
                                             start=(j == 0), stop=(j == last))
                        nc.vector.tensor_mul(att[h][:, qsl],
                                             ao, rcs.pop((qb, h)))

                    dns = [lambda h=h: dn_chunk(h) for h in range(NH)]
                    aos = [lambda h=h: ao_chunk(h) for h in range(NH)]
                    return [dns[0], dns[1], aos[0], dns[2], aos[1], dns[3],
                            aos[2], aos[3]]

                def p3_chunk(tt):
                    """output projection for one 128-row t-tile."""
                    def run():
                        tsl = slice(128 * tt, 128 * (tt + 1))
                        ob = p3s.tile([128, D], bf, tag="ob", name="ob")
                        for dc in range(4):
                            csl = slice(512 * dc, 512 * (dc + 1))
                            ps = p3ps.tile([128, 512], f32, tag="o", name="o")
                            for h in range(NH):
                                nc.tensor.matmul(ps, att[h][:, tsl],
                                                 w_o_sb[:, h * D + 512 * dc:
                                                        h * D + 512 * (dc + 1)],
                                                 start=(h == 0), stop=(h == 3))
                            # split PSUM->SBUF staging between Vector and the
                            # mostly-idle Scalar engine: a lone Vector gets
                            # cast-bound against the PE here
                            if dc % 2 == 0:
                                nc.vector.tensor_copy(out=ob[:, csl], in_=ps)
                            else:
                                nc.scalar.activation(
                                    out=ob[:, csl], in_=ps,
                                    func=mybir.ActivationFunctionType.Copy)
                        nc.sync.dma_start(out=out[tsl, :], in_=ob)
                    return run

                def _blk_tiles(qb):
                    # A1 depends on the strided full-T k tensors (last P1
                    # slice); emit it last so early score matmuls don't stall.
                    blk = [t_ for t_ in _MASK_TILES[qb] if t_["cls"] != "skip"]
                    return ([t_ for t_ in blk if t_["kind"] != "A1"]
                            + [t_ for t_ in blk if t_["kind"] == "A1"])

                def _load_masks(qb):
                    """kick the mask DMAs for block qb (prefetched one ahead)."""
                    blk = _blk_tiles(qb)
                    msk = {}
                    if any(t_["kind"] == "B" and t_["cls"] == "mask" for t_ in blk):
                        mbs = mk.tile([128, NBW * QB], bf, tag="mbs", name="mbs")
                        nc.sync.dma_start(
                            out=mbs.rearrange("p (n m) -> p n m", m=QB),
                            in_=mB[qb].rearrange("n p m -> p n m"))
                        for t_ in blk:
                            if t_["kind"] == "B" and t_["cls"] == "mask":
                                msk[("B", t_["i"])] = mbs[:, t_["i"] * QB:
                                                          (t_["i"] + 1) * QB]
                    for t_ in blk:
                        if t_["cls"] != "mask" or t_["kind"] == "B":
                            continue
                        rows = t_["m"].shape[0]
                        mt = mk.tile([rows, QB], bf, tag=f"m{t_['kind']}",
                                     name=f"m{t_['kind']}")
                        nc.sync.dma_start(
                            out=mt, in_=mA0 if t_["kind"] == "A0" else mA1[qb])
                        msk[(t_["kind"], t_["i"])] = mt
                    return msk

                work = []
                msk_next = _load_masks(0)
                for qb in range(NB):
                    q0 = qb * QB
                    qsl = slice(q0, q0 + QB)
                    blk = _blk_tiles(qb)
                    msk = msk_next
                    if qb + 1 < NB:
                        msk_next = _load_masks(qb + 1)
                    # pass 1: scores + exp + mask, all heads, one key tile at
                    # a time; deferred pass-2/P3 chunks of earlier blocks are
                    # interleaved to keep the PE busy while ACT runs the exps.
                    # Rope matmuls of a head pair are adjacent: disjoint row
                    # groups run concurrently on the PE.
                    exps = {}
                    for jt, t_ in enumerate(blk):
                        kind, i, k0 = t_["kind"], t_["i"], t_["k0"]
                        rows = t_["m"].shape[0]
                        a, b = t_["qr"]
                        w = b - a
                        qv = slice(q0 + a, q0 + b)
                        if kind == "A1":
                            # 32-key dilated tile: pack the 4 heads into
                            # disjoint 32-wide column groups of the PE array —
                            # the 8 matmuls run concurrently on sub-arrays
                            psA = scps.tile([128, QB], f32, tag="sc", name="sc")
                            for h in range(NH):
                                nc.tensor.matmul(
                                    psA[32 * h:32 * h + 32, 0:w],
                                    k_cT[h][:, 0:T:STRIDE], q_cT[h][:, qv],
                                    start=True, stop=False,
                                    tile_position=(0, 32 * h))
                            for h in range(NH):
                                pr, po = h // 2, (h % 2) * 64
                                nc.tensor.matmul(
                                    psA[32 * h:32 * h + 32, 0:w],
                                    k_rT[pr][po:po + 64, 0:T:STRIDE],
                                    q_pad[h][po:po + 64, qv],
                                    start=False, stop=True,
                                    tile_position=(po, 32 * h))
                            pss = [psA[32 * h:32 * h + 32, :] for h in range(NH)]
                        else:
                            # rope matmuls run full-row (128-contraction): the
                            # pair-stacked k_rT is the stationary and q_pad's
                            # zero rows cancel the partner head, so LDWEIGHTS
                            # background pull-ahead keeps the PE back-to-back
                            pss = []
                            for h in range(NH):
                                if kind == "A0":
                                    lk = k_cT[h][:, 0:GLOB]
                                else:
                                    ks = slice(k0 + 128 * i, k0 + 128 * (i + 1))
                                    lk = k_cT[h][:, ks]
                                ps = scps.tile([128, QB], f32, tag="sc", name="sc")
                                nc.tensor.matmul(ps[0:rows, 0:w], lk, q_cT[h][:, qv],
                                                 start=True, stop=False)
                                pss.append(ps)
                            for h in range(NH):
                                pr = h // 2
                                if kind == "A0":
                                    lr = k_rT[pr][:, 0:GLOB]
                                else:
                                    ks = slice(k0 + 128 * i, k0 + 128 * (i + 1))
                                    lr = k_rT[pr][:, ks]
                                nc.tensor.matmul(pss[h][0:rows, 0:w], lr,
                                                 q_pad[h][:, qv],
                                                 start=False, stop=True)
                        for h in range(NH):
                            # elementwise chain stays on Vector: GpSimd was
                            # measured ~8x slower per op (~0.8us fixed cost)
                            # and its latency gates the denominator chain
                            ex = expp.tile([128, QB], bf, tag="ex", name="ex")
                            nc.scalar.activation(out=ex[0:rows, 0:w],
                                                 in_=pss[h][0:rows, 0:w], func=EXP)
                            if t_["cls"] == "mask":
                                nc.vector.tensor_mul(ex[0:rows, 0:w],
                                                     ex[0:rows, 0:w],
                                                     msk[(kind, i)][:, a:b])
                            exps[(h, kind, i)] = (rows, ex)
                            # accumulate the softmax denominator on Vector so
                            # pass 2 needs a single ones-matmul per head
                            if jt == 0:
                                acc = dacc.tile([128, QB], bf, tag="acc",
                                                name="acc")
                                nc.vector.tensor_copy(out=acc, in_=ex)
                                accs[(qb, h)] = acc
                            else:
                                acc = accs[(qb, h)]
                                nc.vector.tensor_add(acc[0:rows, a:b],
                                                     acc[0:rows, a:b],
                                                     ex[0:rows, 0:w])
                        npop = 3 if len(work) > 10 else 2 if len(work) > 8 else 1
                        for _ in range(npop):
                            if work:
                                work.pop(0)[1]()
                    work.extend(("p2", c) for c in p2_chunks(qb, exps))
                    if qb >= 1:
                        work.extend(("p3", p3_chunk(tt)) for tt in
                                    range(4 * (qb - 1), 4 * qb))
                # tail drain: run remaining p3 (pure-PE, inputs ready) chunks
                # between the last block's dn -> reciprocal -> ao chains so
                # the in-order PE queue never stalls on the Vector engine
                p2w = [c for k, c in work if k == "p2"]
                p3w = [c for k, c in work if k == "p3"]
                if p3w:
                    p3w.pop(0)()            # pure PE while Vector flushes the
                                            # last pass-1 exp/acc backlog
                for ch in p2w[:2]:          # dn0, dn1
                    ch()
                if p3w:
                    p3w.pop(0)()            # covers the reciprocals
                for ch in p2w[2:]:          # ao/dn interleaved
                    ch()
                for ch in p3w:
                    ch()
                for tt in range(4 * (NB - 1), 4 * NB):
                    p3_chunk(tt)()
    nc.compile()
    return nc


_NC = None


def _get_nc():
    global _NC
    if _NC is None:
        _NC = _build_program()
    return _NC


def _prep_in_maps(inputs):
    x = np.asarray(inputs["x"], np.float32)
    w_q = np.asarray(inputs["w_q"], np.float32)
    w_dkv = np.asarray(inputs["w_dkv"], np.float32)
    w_uk = np.asarray(inputs["w_uk"], np.float32)
    w_uv = np.asarray(inputs["w_uv"], np.float32)
    w_qp = np.asarray(inputs["w_q_pos"], np.float32)
    w_kp = np.asarray(inputs["w_k_pos"], np.float32)
    w_o = np.asarray(inputs["w_o"], np.float32)

    invf = _inv_freq()                                # [32]
    t = np.arange(T, dtype=np.float32)
    ang = t[None, :] * invf[:, None]                  # [32, T]
    cos32 = np.cos(ang)
    sin32 = np.sin(ang)
    cosT = np.tile(cos32, (4, 1)).astype(np.float32)  # rows p: f = p % 32
    ssgn = np.tile(sin32, (4, 1)).astype(np.float32)
    ssgn[0:32] *= -1.0
    ssgn[64:96] *= -1.0

    mA0 = _MASK_TILES[0][0]["m"].astype(np.float32).astype(BF16)
    mA1 = np.stack([_MASK_TILES[qb][1]["m"] for qb in range(NB)]) \
        .astype(np.float32).astype(BF16)
    mB = np.stack([[_MASK_TILES[qb][2 + i]["m"] for i in range(NBW)]
                   for qb in range(NB)]).astype(np.float32).astype(BF16)

    xT_b = [np.ascontiguousarray(x[b].T).astype(BF16) for b in range(B)]
    common = dict(cosT=cosT, ssgnT=ssgn, mA0=mA0, mA1=mA1, mB=mB,
                  w_dkv=w_dkv.astype(BF16))

    in_maps = []
    for c in range(NCORES):
        b, g = c // 4, c % 4
        ch = slice(4 * g * DH, 4 * (g + 1) * DH)      # content head cols / w_o rows
        rh = slice(4 * g * DR, 4 * (g + 1) * DR)      # rope head cols
        in_maps.append(dict(
            common,
            xT=xT_b[b],
            w_q=(w_q[:, ch] * SCALE).astype(BF16),
            w_uk=np.ascontiguousarray(w_uk[:, ch]).astype(BF16),
            w_uv=np.ascontiguousarray(w_uv[:, ch]).astype(BF16),
            w_qp=(w_qp[:, rh] * (SCALE_ROPE * YARN * YARN)).astype(BF16),
            w_kp=np.ascontiguousarray(w_kp[:, rh]).astype(BF16),
            w_o=np.ascontiguousarray(w_o[ch, :]).astype(BF16),
        ))
    return in_maps


def _run(inputs, trace=False, trace_kwargs=None):
    nc = _get_nc()
    in_maps = _prep_in_maps(inputs)
    res = bass_utils.run_bass_kernel_spmd(
        nc, in_maps, core_ids=list(range(NCORES)), trace=trace,
        **(trace_kwargs or {}))
    out = np.zeros((B, T, D), np.float32)
    for c in range(NCORES):
        out[c // 4] += res.results[c]["out"].astype(np.float32)
    return out, res


def kernel(**inputs) -> np.ndarray:
    out, _ = _run(inputs)
    return out



# revision 16
# speedup vs baseline: 1.0408x; 1.0408x over previous
"""DeepSeek sparse attention (MLA + YaRN RoPE + local/dilated/global mask) on 8 TRN2 cores.

Sharding: (batch, head-group) across 8 cores — core c handles batch c//4, heads
[4*(c%4), 4*(c%4)+4).  Each core computes its projections from the full x (host
pre-transposes x per batch), runs block-sparse attention for its 4 heads, and
produces a row-parallel partial of out @ w_o.  Host sums the 4 partials per batch.

Each core computes the full c_kv latent redundantly: deduplicating it via a
group AllGather was measured to put the chip in a lower power state (~2.0 GHz
PE clock for any collective-enabled NEFF vs 2.4 GHz without), which costs far
more than the duplicated matmuls save.

Layout: "transposed" activations [feature, t] so every matmul keeps the moving
operand in the free dim (N=512/256) at full bf16 rate and no on-chip transposes
are needed anywhere.  Scores are computed as S^T[k, q]; exp tiles are summed on
the Vector engine into one [128, QB] accumulator per (head, block) so the
softmax denominator costs a single ones-matmul; the divide is an elementwise
mul by the reciprocal.
"""

import sys

if "/opt/trn_rl_repo" not in sys.path:
    sys.path.insert(0, "/opt/trn_rl_repo")

import ml_dtypes
import numpy as np

import concourse.bass as bass  # noqa: F401  (bass types used via tile/bacc)
import concourse.mybir as mybir
import concourse.tile as tile
from concourse import bacc, bass_utils

BF16 = ml_dtypes.bfloat16

# ---- problem constants (hardcoded per contract) ----
B, T, D = 2, 2048, 2048
H, DH, DR, DL = 16, 128, 64, 512
WINDOW, STRIDE, GLOB = 512, 64, 128
BASE, MAX_SEQ, ORIG_MAX = 10000.0, 131072, 4096
BETA_FAST, BETA_SLOW = 32.0, 1.0
SCALE = 1.0 / float(np.sqrt(DH))
SCALE_ROPE = 1.0 / float(np.sqrt(DR))
YARN = float(np.float32(0.1 * np.log(MAX_SEQ / ORIG_MAX) + 1.0))
HALF = WINDOW // 2

NCORES = 8
NH = 4            # heads per core
NP = 2            # head-pairs per core (rope tiles stack 2 heads on 128 partitions)
QB = 512          # query block
NB = T // QB      # 4
NBW = (HALF + QB) // 128   # window-strip tiles per block
SL = 512          # t-slice width in projection phase
NS = T // SL      # 4
NT = T // 128     # 16
DLC = DL // 4     # latent dims computed per core (128)
NWARM = 48        # warm-up matmuls: keep PE busy+warm during the input DMA


def _inv_freq():
    base_inv = 1.0 / (BASE ** (np.arange(0, DR, 2, dtype=np.float32) / DR))
    scale = MAX_SEQ / ORIG_MAX
    freqs = np.arange(DR // 2, dtype=np.float32)
    ramp = np.clip((freqs - BETA_SLOW) / (BETA_FAST - BETA_SLOW), 0.0, 1.0)
    return (base_inv * (1 - ramp) + (base_inv / scale) * ramp).astype(np.float32)


def _full_mask():
    pos = np.arange(T)
    qp, kp = pos[:, None], pos[None, :]
    dist = qp - kp
    window = (dist >= -HALF) & (dist <= HALF)
    dil = (kp % STRIDE == 0) | (kp < GLOB)
    return (window | dil) & (kp <= qp)


def _mask_tiles():
    """Per q-block key tiles, with exactly-once ownership masks.

    Tiles: A0 = keys [0, 128) (global), A1 = 32 dilated keys {64j}, B0..B3 =
    the 512-wide sliding window strip.  A0 owns k<128; A1 owns k%64==0 & k>=128;
    B owns the rest.  Each tile classified: 'skip' (all-zero), 'ones', 'mask'.
    """
    full = _full_mask()
    blocks = []
    for qb in range(NB):
        q0 = qb * QB
        k0 = max(0, q0 - HALF)
        qs = slice(q0, q0 + QB)
        blk = []
        m = full[qs, 0:GLOB].T.copy()                        # [128, QB]
        blk.append(dict(kind="A0", i=0, k0=0, keys=np.arange(GLOB), m=m))
        keys = np.arange(32) * STRIDE
        m = full[qs, :][:, keys].T.copy()                    # [32, QB]
        m[keys < GLOB] = False
        blk.append(dict(kind="A1", i=0, k0=0, keys=keys, m=m))
        for i in range(NBW):
            kk = k0 + 128 * i + np.arange(128)
            m = full[qs, :][:, kk].T.copy()
            m[(kk < GLOB) | (kk % STRIDE == 0)] = False
            blk.append(dict(kind="B", i=i, k0=k0, keys=kk, m=m))
        for t_ in blk:
            t_["cls"] = ("skip" if not t_["m"].any()
                         else "ones" if t_["m"].all() else "mask")
            if t_["cls"] == "skip":
                t_["qr"] = (0, QB)
            else:
                cols = np.flatnonzero(t_["m"].any(axis=0))
                a, b = int(cols[0]), int(cols[-1]) + 1
                assert (t_["m"].any(axis=0)[a:b]).all()  # contiguous
                t_["qr"] = (a, b)
        # accumulation groups start with tile 0: it must span all q columns
        assert blk[0]["qr"] == (0, QB)
        blocks.append(blk)
    # exactly-once coverage check against the reference mask
    for qb in range(NB):
        cov = np.zeros((QB, T), dtype=np.int32)
        for t_ in blocks[qb]:
            cov[np.arange(QB)[:, None], t_["keys"][None, :]] += t_["m"].T
        assert (cov == full[qb * QB:(qb + 1) * QB].astype(np.int32)).all()
    return blocks


_MASK_TILES = _mask_tiles()


def _build_program():
    nc = bacc.Bacc("TRN2", target_bir_lowering=False, debug=False,
                   enable_asserts=False, num_devices=NCORES)
    bf, f32 = mybir.dt.bfloat16, mybir.dt.float32

    xT = nc.dram_tensor("xT", [D, T], bf, kind="ExternalInput").ap()
    w_q = nc.dram_tensor("w_q", [D, NH * DH], bf, kind="ExternalInput").ap()
    w_dkv = nc.dram_tensor("w_dkv", [D, DL], bf, kind="ExternalInput").ap()
    w_uk = nc.dram_tensor("w_uk", [DL, NH * DH], bf, kind="ExternalInput").ap()
    w_uv = nc.dram_tensor("w_uv", [DL, NH * DH], bf, kind="ExternalInput").ap()
    w_qp = nc.dram_tensor("w_qp", [D, NH * DR], bf, kind="ExternalInput").ap()
    w_kp = nc.dram_tensor("w_kp", [D, NH * DR], bf, kind="ExternalInput").ap()
    w_o = nc.dram_tensor("w_o", [NH * DH, D], bf, kind="ExternalInput").ap()
    cosT = nc.dram_tensor("cosT", [128, T], f32, kind="ExternalInput").ap()
    ssgnT = nc.dram_tensor("ssgnT", [128, T], f32, kind="ExternalInput").ap()
    mA0 = nc.dram_tensor("mA0", [GLOB, QB], bf, kind="ExternalInput").ap()
    mA1 = nc.dram_tensor("mA1", [NB, 32, QB], bf, kind="ExternalInput").ap()
    mB = nc.dram_tensor("mB", [NB, NBW, 128, QB], bf, kind="ExternalInput").ap()
    out = nc.dram_tensor("out", [T, D], bf, kind="ExternalOutput").ap()

    EXP = mybir.ActivationFunctionType.Exp

    with tile.TileContext(nc) as tc:
        with tc.tile_pool(name="acts", bufs=1) as acts, \
             tc.tile_pool(name="consts", bufs=1) as consts:
            q_cT = [acts.tile([128, T], bf, tag=f"q_cT{h}", name=f"q_cT{h}") for h in range(NH)]
            k_cT = [acts.tile([128, T], bf, tag=f"k_cT{h}", name=f"k_cT{h}") for h in range(NH)]
            # per-head q_rope tiles: head h's 64 rope dims live in rows
            # po..po+64 (po = (h%2)*64); the other 64 rows are ZERO so the
            # score matmul can run full-row (128-contraction) against the
            # pair-stacked k_rT stationary -- zeros kill the partner head's
            # contribution.  Full-row MMs keep LDWEIGHTS pull-ahead alive
            # (row_grp MMs were measured to serialize with ~250ns bubbles).
            q_pad = [acts.tile([128, T], bf, tag=f"q_pad{h}", name=f"q_pad{h}") for h in range(NH)]
            k_rT = [acts.tile([128, T], bf, tag=f"k_rT{p}", name=f"k_rT{p}") for p in range(NP)]
            v_t = [acts.tile([128, NH * DH], bf, tag=f"v{t_}", name=f"v{t_}") for t_ in range(NT)]
            v_A = acts.tile([32, NH * DH], bf, tag="v_A", name="v_A")
            ones = consts.tile([128, 128], bf, tag="ones", name="ones")
            nc.vector.memset(ones, 1.0)
            junk = consts.tile([128, 128], bf, tag="junk", name="junk")
            nc.vector.memset(junk, 0.001)
            # zero the pad halves on the (otherwise idle) GpSimd engine so the
            # Vector queue stays clear for the warm-up gating memsets above
            for h in range(NH):
                po = (h % 2) * 64
                nc.gpsimd.memset(q_pad[h][64 - po:128 - po, :], 0.0)
            # load the Exp LUT into ACT early: the first real exp otherwise
            # pays a 1.3us table load right when P2's PSUM rotation is tight
            warm = consts.tile([1, 2], f32, tag="warm", name="warm")
            nc.vector.memset(warm, 0.0)
            nc.scalar.activation(out=warm, in_=warm, func=EXP)

            # ---------------- P1: projections (t-slice streamed) --------------
            with tc.tile_pool(name="wp1", bufs=1) as wp1, \
                 tc.tile_pool(name="xch", bufs=2) as xch, \
                 tc.tile_pool(name="ckvp", bufs=1) as ckvp, \
                 tc.tile_pool(name="rope_t", bufs=3) as rope_t, \
                 tc.tile_pool(name="p1ps", bufs=7, space="PSUM") as p1ps, \
                 tc.tile_pool(name="vAps", bufs=1, space="PSUM") as vAps:
                # full c_kv^T latent (4 chunks of 128 dims x T), P1-scoped
                ckv = [ckvp.tile([128, T], bf, tag=f"ckv{g}", name=f"ckv{g}")
                       for g in range(4)]
                # warm-up matmuls: no data deps, so they issue from t=0 and
                # keep the PE busy (and the HAM clock un-throttled) while the
                # first x/weight DMAs land
                wu = p1ps.tile([128, SL], f32, tag="proj", name="wu")
                for _ in range(NWARM):
                    nc.tensor.matmul(wu[:, 0:128], ones, junk,
                                     start=True, stop=True)

                # 4-d-tile chunks for w_q and x: Tile's DMA dependency is
                # tile-granular, so chunked tiles let the d=0 matmuls start
                # after ~1/4 of the load.  Chunks are kept coarse (512KB)
                # because every dma_start costs ~650ns of serialized issue
                # time on the Sync engine.
                wq_c = [wp1.tile([128, 4 * NH * DH], bf, tag=f"wq{c}",
                                 name=f"wq{c}") for c in range(4)]
                x0_c = [xch.tile([128, 4 * SL], bf, tag=f"xc{c}",
                                 name=f"xc{c}") for c in range(4)]
                wdkv_sb = wp1.tile([128, 16 * DL], bf, tag="wdkv", name="wdkv")
                wqp_sb = wp1.tile([128, 16 * NH * DR], bf, tag="wqp", name="wqp")
                wkp_sb = wp1.tile([128, 16 * NH * DR], bf, tag="wkp", name="wkp")
                wuk_sb = wp1.tile([128, 4 * NH * DH], bf, tag="wuk", name="wuk")
                wuv_sb = wp1.tile([128, 4 * NH * DH], bf, tag="wuv", name="wuv")
                cos_sb = wp1.tile([128, T], f32, tag="cos", name="cos")
                ssg_sb = wp1.tile([128, T], f32, tag="ssg", name="ssg")

                def _wslice(big, cols, d, c0, c1):
                    return big[:, d * cols + c0:d * cols + c1]

                def _load_w(dst, src, cols):
                    nc.sync.dma_start(
                        out=dst.rearrange("p (n m) -> p n m", m=cols),
                        in_=src.rearrange("(n p) m -> p n m", p=128))

                # ordered so slice-0 compute can start ASAP: interleave the
                # x / w_q chunks first (consumed in d order), then the
                # weights in the order the schedule needs them
                wq_r = w_q.rearrange("(n p) m -> p n m", p=128)
                xT_r = xT.rearrange("(n p) m -> p n m", p=128)

                def _load_xc(dst, s, c):
                    nc.sync.dma_start(
                        out=dst.rearrange("p (n m) -> p n m", m=SL),
                        in_=xT_r[:, 4 * c:4 * (c + 1), s * SL:(s + 1) * SL])

                for c in range(4):
                    _load_xc(x0_c[c], 0, c)
                    nc.sync.dma_start(
                        out=wq_c[c].rearrange("p (n m) -> p n m", m=NH * DH),
                        in_=wq_r[:, 4 * c:4 * (c + 1)])
                _load_w(wdkv_sb, w_dkv, DL)
                _load_w(wqp_sb, w_qp, NH * DR)
                nc.sync.dma_start(out=cos_sb, in_=cosT)
                nc.sync.dma_start(out=ssg_sb, in_=ssgnT)
                _load_w(wkp_sb, w_kp, NH * DR)
                _load_w(wuk_sb, w_uk, NH * DH)
                _load_w(wuv_sb, w_uv, NH * DH)

                COPY = mybir.ActivationFunctionType.Copy
                xcur = x0_c
                for s in range(NS):
                    t0 = s * SL
                    tsl = slice(t0, t0 + SL)
                    xnxt = None
                    if s + 1 < NS:  # prefetch next slice (4-d chunk tiles)
                        xnxt = [xch.tile([128, 4 * SL], bf, tag=f"xc{c}",
                                         name=f"xc{c}") for c in range(4)]
                        for c in range(4):
                            _load_xc(xnxt[c], s + 1, c)
                    xt = [xcur[d // 4][:, (d % 4) * SL:(d % 4 + 1) * SL]
                          for d in range(16)]
                    # content q projections, d-outer so the first matmuls only
                    # need the first DMA chunk of x/w_q (fast start on slice 0)
                    qps = [p1ps.tile([128, SL], f32, tag="proj", name="proj")
                           for _ in range(NH)]
                    for d in range(16):
                        for h in range(NH):
                            nc.tensor.matmul(
                                qps[h],
                                _wslice(wq_c[d // 4], NH * DH, d % 4,
                                        h * DH, (h + 1) * DH),
                                xt[d], start=(d == 0), stop=(d == 15))
                    for h in range(NH):
                        nc.vector.tensor_copy(out=q_cT[h][:, tsl], in_=qps[h])
                    # full c_kv latent for this slice, into the persistent
                    # ckv tiles (4 chunks of 128 latent dims)
                    for g in range(4):
                        ps = p1ps.tile([128, SL], f32, tag="proj", name="proj")
                        for d in range(16):
                            nc.tensor.matmul(
                                ps, _wslice(wdkv_sb, DL, d, g * 128, (g + 1) * 128),
                                xt[d], start=(d == 0), stop=(d == 15))
                        nc.vector.tensor_copy(out=ckv[g][:, tsl], in_=ps)
                    # rope projections + rotation (pair-stacked: 2 heads / tile)
                    for w_sb, isq in ((wqp_sb, True), (wkp_sb, False)):
                        for p in range(NP):
                            ps = p1ps.tile([128, SL], f32, tag="proj", name="proj")
                            for d in range(16):
                                nc.tensor.matmul(
                                    ps, _wslice(w_sb, NH * DR, d, p * 128, (p + 1) * 128),
                                    xt[d], start=(d == 0), stop=(d == 15))
                            m1 = rope_t.tile([128, SL], bf, tag="m1", name="m1")
                            nc.vector.tensor_mul(m1, ps, cos_sb[:, tsl])
                            m2 = rope_t.tile([128, SL], bf, tag="m2", name="m2")
                            for a in (0, 32, 64, 96):
                                sw = a ^ 32
                                nc.vector.tensor_mul(m2[a:a + 32, :],
                                                     ps[sw:sw + 32, :],
                                                     ssg_sb[a:a + 32, tsl])
                            if isq:
                                # split into the per-head zero-padded tiles
                                nc.vector.tensor_add(
                                    q_pad[2 * p][0:64, tsl],
                                    m1[0:64, :], m2[0:64, :])
                                nc.vector.tensor_add(
                                    q_pad[2 * p + 1][64:128, tsl],
                                    m1[64:128, :], m2[64:128, :])
                            else:
                                nc.vector.tensor_add(k_rT[p][:, tsl], m1, m2)
                    xcur = xnxt

                # k content + v from the latent; PSUM->SBUF staging on the
                # Scalar engine here -- Vector must be drained by the time the
                # first attention block's exp/acc chain starts, or the early
                # score matmuls stall on PSUM recycling
                for s in range(NS):
                    t0 = s * SL
                    tsl = slice(t0, t0 + SL)
                    for h in range(NH):
                        ps = p1ps.tile([128, SL], f32, tag="proj", name="proj")
                        for g in range(4):
                            nc.tensor.matmul(
                                ps, _wslice(wuk_sb, NH * DH, g, h * DH, (h + 1) * DH),
                                ckv[g][:, tsl], start=(g == 0), stop=(g == 3))
                        nc.scalar.activation(out=k_cT[h][:, tsl], in_=ps,
                                             func=COPY)
                    # v in [t, dh] layout
                    for u in range(4):
                        usl = slice(t0 + u * 128, t0 + (u + 1) * 128)
                        ps = p1ps.tile([128, NH * DH], f32, tag="proj", name="proj")
                        for g in range(4):
                            nc.tensor.matmul(
                                ps, ckv[g][:, usl],
                                wuv_sb[:, g * NH * DH:(g + 1) * NH * DH],
                                start=(g == 0), stop=(g == 3))
                        nc.vector.tensor_copy(out=v_t[s * 4 + u], in_=ps)
                # dilated-key V rows (keys 64j), one pass over full T
                psA = vAps.tile([32, NH * DH], f32, tag="vA", name="vA")
                for g in range(4):
                    nc.tensor.matmul(psA, ckv[g][:, 0:T:STRIDE],
                                     wuv_sb[:, g * NH * DH:(g + 1) * NH * DH],
                                     start=(g == 0), stop=(g == 3))
                nc.vector.tensor_copy(out=v_A, in_=psA)

            # ---------------- P2: block-sparse attention ----------------------
            with tc.tile_pool(name="wo", bufs=1) as wo, \
                 tc.tile_pool(name="attp", bufs=1) as attp:
              att = [attp.tile([128, T], bf, tag=f"att{h}", name=f"att{h}")
                     for h in range(NH)]
              w_o_sb = wo.tile([128, NH * D], bf, tag="wo", name="wo")
              nc.sync.dma_start(
                  out=w_o_sb.rearrange("p (n m) -> p n m", m=D),
                  in_=w_o.rearrange("(n p) m -> p n m", p=128))

              with tc.tile_pool(name="mk", bufs=2) as mk, \
                   tc.tile_pool(name="exp", bufs=48) as expp, \
                   tc.tile_pool(name="dacc", bufs=12) as dacc, \
                   tc.tile_pool(name="p2t", bufs=3) as p2t, \
                   tc.tile_pool(name="p3s", bufs=4) as p3s, \
                   tc.tile_pool(name="scps", bufs=5, space="PSUM") as scps, \
                   tc.tile_pool(name="daops", bufs=1, space="PSUM") as daops, \
                   tc.tile_pool(name="p3ps", bufs=2, space="PSUM") as p3ps:
                rcs = {}
                accs = {}

                def p2_chunks(qb, exps):
                    """pass 2 of block qb as deferred emitters (2 per head).

                    Enqueued as dn0, dn1, ao0, dn2, ao1, dn3, ao2, ao3 so a
                    head's reciprocal (Vector) runs under the next head's dn /
                    interleaved score matmuls instead of stalling the in-order
                    PE queue.
                    """
                    q0 = qb * QB
                    qsl = slice(q0, q0 + QB)
                    blk = [t_ for t_ in _MASK_TILES[qb] if t_["cls"] != "skip"]
                    last = len(blk) - 1

                    def dn_chunk(h):
                        dn = scps.tile([128, QB], f32, tag="sc", name="dn")
                        nc.tensor.matmul(dn, ones, accs.pop((qb, h)),
                                         start=True, stop=True)
                        rc = p2t.tile([128, QB], f32, tag="rc", name="rc")
                        nc.vector.reciprocal_approx_fast(out=rc, in_=dn)
                        rcs[(qb, h)] = rc

                    def ao_chunk(h):
                        hs = slice(h * DH, (h + 1) * DH)
                        ao = daops.tile([128, QB], f32, tag="dao", name="ao")
                        for j, t_ in enumerate(blk):
                            kind, i, k0 = t_["kind"], t_["i"], t_["k0"]
                            rows, ex = exps[(h, kind, i)]
                            if kind == "A0":
                                lv = v_t[0][:, hs]
                            elif kind == "A1":
                                lv = v_A[:, hs]
                            else:
                                lv = v_t[k0 // 128 + i][:, hs]
                            a, b = t_["qr"]
                            nc.tensor.matmul(ao[:, a:b], lv[0:rows, :],
                                             ex[0:rows, 0:b - a],
                                             start=(j == 0), stop=(j == last))
                        nc.vector.tensor_mul(att[h][:, qsl],
                                             ao, rcs.pop((qb, h)))

                    dns = [lambda h=h: dn_chunk(h) for h in range(NH)]
                    aos = [lambda h=h: ao_chunk(h) for h in range(NH)]
                    return [dns[0], dns[1], aos[0], dns[2], aos[1], dns[3],
                            aos[2], aos[3]]

                def p3_chunk(tt):
                    """output projection for one 128-row t-tile."""
                    def run():
                        tsl = slice(128 * tt, 128 * (tt + 1))
                        ob = p3s.tile([128, D], bf, tag="ob", name="ob")
                        for dc in range(4):
                            csl = slice(512 * dc, 512 * (dc + 1))
                            ps = p3ps.tile([128, 512], f32, tag="o", name="o")
                            for h in range(NH):
                                nc.tensor.matmul(ps, att[h][:, tsl],
                                                 w_o_sb[:, h * D + 512 * dc:
                                                        h * D + 512 * (dc + 1)],
                                                 start=(h == 0), stop=(h == 3))
                            # split PSUM->SBUF staging between Vector and the
                            # mostly-idle Scalar engine: a lone Vector gets
                            # cast-bound against the PE here
                            if dc % 2 == 0:
                                nc.vector.tensor_copy(out=ob[:, csl], in_=ps)
                            else:
                                nc.scalar.activation(
                                    out=ob[:, csl], in_=ps,
                                    func=mybir.ActivationFunctionType.Copy)
                        nc.sync.dma_start(out=out[tsl, :], in_=ob)
                    return run

                def _blk_tiles(qb):
                    # A1 depends on the strided full-T k tensors (last P1
                    # slice); emit it last so early score matmuls don't stall.
                    blk = [t_ for t_ in _MASK_TILES[qb] if t_["cls"] != "skip"]
                    return ([t_ for t_ in blk if t_["kind"] != "A1"]
                            + [t_ for t_ in blk if t_["kind"] == "A1"])

                def _load_masks(qb):
                    """kick the mask DMAs for block qb (prefetched one ahead)."""
                    blk = _blk_tiles(qb)
                    msk = {}
                    if any(t_["kind"] == "B" and t_["cls"] == "mask" for t_ in blk):
                        mbs = mk.tile([128, NBW * QB], bf, tag="mbs", name="mbs")
                        nc.sync.dma_start(
                            out=mbs.rearrange("p (n m) -> p n m", m=QB),
                            in_=mB[qb].rearrange("n p m -> p n m"))
                        for t_ in blk:
                            if t_["kind"] == "B" and t_["cls"] == "mask":
                                msk[("B", t_["i"])] = mbs[:, t_["i"] * QB:
                                                          (t_["i"] + 1) * QB]
                    for t_ in blk:
                        if t_["cls"] != "mask" or t_["kind"] == "B":
                            continue
                        rows = t_["m"].shape[0]
                        mt = mk.tile([rows, QB], bf, tag=f"m{t_['kind']}",
                                     name=f"m{t_['kind']}")
                        nc.sync.dma_start(
                            out=mt, in_=mA0 if t_["kind"] == "A0" else mA1[qb])
                        msk[(t_["kind"], t_["i"])] = mt
                    return msk

                work = []
                msk_next = _load_masks(0)
                for qb in range(NB):
                    q0 = qb * QB
                    qsl = slice(q0, q0 + QB)
                    blk = _blk_tiles(qb)
                    msk = msk_next
                    if qb + 1 < NB:
                        msk_next = _load_masks(qb + 1)
                    # pass 1: scores + exp + mask, all heads, one key tile at
                    # a time; deferred pass-2/P3 chunks of earlier blocks are
                    # interleaved to keep the PE busy while ACT runs the exps.
                    # Rope matmuls of a head pair are adjacent: disjoint row
                    # groups run concurrently on the PE.
                    exps = {}
                    for jt, t_ in enumerate(blk):
                        kind, i, k0 = t_["kind"], t_["i"], t_["k0"]
                        rows = t_["m"].shape[0]
                        a, b = t_["qr"]
                        w = b - a
                        qv = slice(q0 + a, q0 + b)
                        if kind == "A1":
                            # 32-key dilated tile: pack the 4 heads into
                            # disjoint 32-wide column groups of the PE array —
                            # the 8 matmuls run concurrently on sub-arrays
                            psA = scps.tile([128, QB], f32, tag="sc", name="sc")
                            for h in range(NH):
                                nc.tensor.matmul(
                                    psA[32 * h:32 * h + 32, 0:w],
                                    k_cT[h][:, 0:T:STRIDE], q_cT[h][:, qv],
                                    start=True, stop=False,
                                    tile_position=(0, 32 * h))
                            for h in range(NH):
                                pr, po = h // 2, (h % 2) * 64
                                nc.tensor.matmul(
                                    psA[32 * h:32 * h + 32, 0:w],
                                    k_rT[pr][po:po + 64, 0:T:STRIDE],
                                    q_pad[h][po:po + 64, qv],
                                    start=False, stop=True,
                                    tile_position=(po, 32 * h))
                            pss = [psA[32 * h:32 * h + 32, :] for h in range(NH)]
                        else:
                            # rope matmuls run full-row (128-contraction): the
                            # pair-stacked k_rT is the stationary and q_pad's
                            # zero rows cancel the partner head, so LDWEIGHTS
                            # background pull-ahead keeps the PE back-to-back
                            pss = []
                            for h in range(NH):
                                if kind == "A0":
                                    lk = k_cT[h][:, 0:GLOB]
                                else:
                                    ks = slice(k0 + 128 * i, k0 + 128 * (i + 1))
                                    lk = k_cT[h][:, ks]
                                ps = scps.tile([128, QB], f32, tag="sc", name="sc")
                                nc.tensor.matmul(ps[0:rows, 0:w], lk, q_cT[h][:, qv],
                                                 start=True, stop=False)
                                pss.append(ps)
                            for h in range(NH):
                                pr = h // 2
                                if kind == "A0":
                                    lr = k_rT[pr][:, 0:GLOB]
                                else:
                                    ks = slice(k0 + 128 * i, k0 + 128 * (i + 1))
                                    lr = k_rT[pr][:, ks]
                                nc.tensor.matmul(pss[h][0:rows, 0:w], lr,
                                                 q_pad[h][:, qv],
                                                 start=False, stop=True)
                        for h in range(NH):
                            # elementwise chain stays on Vector: GpSimd was
                            # measured ~8x slower per op (~0.8us fixed cost)
                            # and its latency gates the denominator chain
                            ex = expp.tile([128, QB], bf, tag="ex", name="ex")
                            nc.scalar.activation(out=ex[0:rows, 0:w],
                                                 in_=pss[h][0:rows, 0:w], func=EXP)
                            if t_["cls"] == "mask":
                                nc.vector.tensor_mul(ex[0:rows, 0:w],
                                                     ex[0:rows, 0:w],
                                                     msk[(kind, i)][:, a:b])
                            exps[(h, kind, i)] = (rows, ex)
                            # accumulate the softmax denominator on Vector so
                            # pass 2 needs a single ones-matmul per head
                            if jt == 0:
                                acc = dacc.tile([128, QB], bf, tag="acc",
                                                name="acc")
                                nc.vector.tensor_copy(out=acc, in_=ex)
                                accs[(qb, h)] = acc
                            else:
                                acc = accs[(qb, h)]
                                nc.vector.tensor_add(acc[0:rows, a:b],
                                                     acc[0:rows, a:b],
                                                     ex[0:rows, 0:w])
                        npop = 3 if len(work) > 10 else 2 if len(work) > 8 else 1
                        for _ in range(npop):
                            if work:
                                work.pop(0)[1]()
                    work.extend(("p2", c) for c in p2_chunks(qb, exps))
                    if qb >= 1:
                        work.extend(("p3", p3_chunk(tt)) for tt in
                                    range(4 * (qb - 1), 4 * qb))
                # tail drain: run remaining p3 (pure-PE, inputs ready) chunks
                # between the last block's dn -> reciprocal -> ao chains so
                # the in-order PE queue never stalls on the Vector engine
                p2w = [c for k, c in work if k == "p2"]
                p3w = [c for k, c in work if k == "p3"]
                if p3w:
                    p3w.pop(0)()            # pure PE while Vector flushes the
                                            # last pass-1 exp/acc backlog
                for ch in p2w[:2]:          # dn0, dn1
                    ch()
                if p3w:
                    p3w.pop(0)()            # covers the reciprocals
                for ch in p2w[2:]:          # ao/dn interleaved
                    ch()
                for ch in p3w:
                    ch()
                for tt in range(4 * (NB - 1), 4 * NB):
                    p3_chunk(tt)()
    nc.compile()
    return nc


_NC = None


def _get_nc():
    global _NC
    if _NC is None:
        _NC = _build_program()
    return _NC


def _prep_in_maps(inputs):
    x = np.asarray(inputs["x"], np.float32)
    w_q = np.asarray(inputs["w_q"], np.float32)
    w_dkv = np.asarray(inputs["w_dkv"], np.float32)
    w_uk = np.asarray(inputs["w_uk"], np.float32)
    w_uv = np.asarray(inputs["w_uv"], np.float32)
    w_qp = np.asarray(inputs["w_q_pos"], np.float32)
    w_kp = np.asarray(inputs["w_k_pos"], np.float32)
    w_o = np.asarray(inputs["w_o"], np.float32)

    invf = _inv_freq()                                # [32]
    t = np.arange(T, dtype=np.float32)
    ang = t[None, :] * invf[:, None]                  # [32, T]
    cos32 = np.cos(ang)
    sin32 = np.sin(ang)
    cosT = np.tile(cos32, (4, 1)).astype(np.float32)  # rows p: f = p % 32
    ssgn = np.tile(sin32, (4, 1)).astype(np.float32)
    ssgn[0:32] *= -1.0
    ssgn[64:96] *= -1.0

    mA0 = _MASK_TILES[0][0]["m"].astype(np.float32).astype(BF16)
    mA1 = np.stack([_MASK_TILES[qb][1]["m"] for qb in range(NB)]) \
        .astype(np.float32).astype(BF16)
    mB = np.stack([[_MASK_TILES[qb][2 + i]["m"] for i in range(NBW)]
                   for qb in range(NB)]).astype(np.float32).astype(BF16)

    xT_b = [np.ascontiguousarray(x[b].T).astype(BF16) for b in range(B)]
    common = dict(cosT=cosT, ssgnT=ssgn, mA0=mA0, mA1=mA1, mB=mB,
                  w_dkv=w_dkv.astype(BF16))

    in_maps = []
    for c in range(NCORES):
        b, g = c // 4, c % 4
        ch = slice(4 * g * DH, 4 * (g + 1) * DH)      # content head cols / w_o rows
        rh = slice(4 * g * DR, 4 * (g + 1) * DR)      # rope head cols
        in_maps.append(dict(
            common,
            xT=xT_b[b],
            w_q=(w_q[:, ch] * SCALE).astype(BF16),
            w_uk=np.ascontiguousarray(w_uk[:, ch]).astype(BF16),
            w_uv=np.ascontiguousarray(w_uv[:, ch]).astype(BF16),
            w_qp=(w_qp[:, rh] * (SCALE_ROPE * YARN * YARN)).astype(BF16),
            w_kp=np.ascontiguousarray(w_kp[:, rh]).astype(BF16),
            w_o=np.ascontiguousarray(w_o[ch, :]).astype(BF16),
        ))
    return in_maps


def _run(inputs, trace=False, trace_kwargs=None):
    nc = _get_nc()
    in_maps = _prep_in_maps(inputs)
    res = bass_utils.run_bass_kernel_spmd(
        nc, in_maps, core_ids=list(range(NCORES)), trace=trace,
        **(trace_kwargs or {}))
    out = np.zeros((B, T, D), np.float32)
    for c in range(NCORES):
        out[c // 4] += res.results[c]["out"].astype(np.float32)
    return out, res


def kernel(**inputs) -> np.ndarray:
    out, _ = _run(inputs)
    return out



# revision 27
# speedup vs baseline: 1.0542x; 1.0128x over previous
"""DeepSeek sparse attention (MLA + YaRN RoPE + local/dilated/global mask) on 8 TRN2 cores.

Sharding: (batch, head-group) across 8 cores — core c handles batch c//4, heads
[4*(c%4), 4*(c%4)+4).  Each core computes its projections from the full x (host
pre-transposes x per batch), runs block-sparse attention for its 4 heads, and
produces a row-parallel partial of out @ w_o.  Host sums the 4 partials per batch.

Each core computes the full c_kv latent redundantly: deduplicating it via a
group AllGather was measured to put the chip in a lower power state (~2.0 GHz
PE clock for any collective-enabled NEFF vs 2.4 GHz without), which costs far
more than the duplicated matmuls save.

Layout: "transposed" activations [feature, t] so every matmul keeps the moving
operand in the free dim (N=512/256) at full bf16 rate and no on-chip transposes
are needed anywhere.  Scores are computed as S^T[k, q]; exp tiles are summed on
the Vector engine into one [128, QB] accumulator per (head, block) so the
softmax denominator costs a single ones-matmul; the divide is an elementwise
mul by the reciprocal.
"""

import sys

if "/opt/trn_rl_repo" not in sys.path:
    sys.path.insert(0, "/opt/trn_rl_repo")

import ml_dtypes
import numpy as np

import concourse.bass as bass  # noqa: F401  (bass types used via tile/bacc)
import concourse.mybir as mybir
import concourse.tile as tile
from concourse import bacc, bass_utils

BF16 = ml_dtypes.bfloat16

# ---- problem constants (hardcoded per contract) ----
B, T, D = 2, 2048, 2048
H, DH, DR, DL = 16, 128, 64, 512
WINDOW, STRIDE, GLOB = 512, 64, 128
BASE, MAX_SEQ, ORIG_MAX = 10000.0, 131072, 4096
BETA_FAST, BETA_SLOW = 32.0, 1.0
SCALE = 1.0 / float(np.sqrt(DH))
SCALE_ROPE = 1.0 / float(np.sqrt(DR))
YARN = float(np.float32(0.1 * np.log(MAX_SEQ / ORIG_MAX) + 1.0))
HALF = WINDOW // 2

NCORES = 8
NH = 4            # heads per core
NP = 2            # head-pairs per core (rope tiles stack 2 heads on 128 partitions)
QB = 512          # query block
NB = T // QB      # 4
NBW = (HALF + QB) // 128   # window-strip tiles per block
SL = 512          # t-slice width in projection phase
NS = T // SL      # 4
NT = T // 128     # 16
DLC = DL // 4     # latent dims computed per core (128)
NWARM = 48        # warm-up matmuls: keep PE busy+warm during the input DMA


def _inv_freq():
    base_inv = 1.0 / (BASE ** (np.arange(0, DR, 2, dtype=np.float32) / DR))
    scale = MAX_SEQ / ORIG_MAX
    freqs = np.arange(DR // 2, dtype=np.float32)
    ramp = np.clip((freqs - BETA_SLOW) / (BETA_FAST - BETA_SLOW), 0.0, 1.0)
    return (base_inv * (1 - ramp) + (base_inv / scale) * ramp).astype(np.float32)


def _full_mask():
    pos = np.arange(T)
    qp, kp = pos[:, None], pos[None, :]
    dist = qp - kp
    window = (dist >= -HALF) & (dist <= HALF)
    dil = (kp % STRIDE == 0) | (kp < GLOB)
    return (window | dil) & (kp <= qp)


def _mask_tiles():
    """Per q-block key tiles, with exactly-once ownership masks.

    Tiles: A0 = keys [0, 128) (global), A1 = 32 dilated keys {64j}, B0..B3 =
    the 512-wide sliding window strip.  A0 owns k<128; A1 owns k%64==0 & k>=128;
    B owns the rest.  Each tile classified: 'skip' (all-zero), 'ones', 'mask'.
    """
    full = _full_mask()
    blocks = []
    for qb in range(NB):
        q0 = qb * QB
        k0 = max(0, q0 - HALF)
        qs = slice(q0, q0 + QB)
        blk = []
        m = full[qs, 0:GLOB].T.copy()                        # [128, QB]
        blk.append(dict(kind="A0", i=0, k0=0, keys=np.arange(GLOB), m=m))
        keys = np.arange(32) * STRIDE
        m = full[qs, :][:, keys].T.copy()                    # [32, QB]
        m[keys < GLOB] = False
        blk.append(dict(kind="A1", i=0, k0=0, keys=keys, m=m))
        for i in range(NBW):
            kk = k0 + 128 * i + np.arange(128)
            m = full[qs, :][:, kk].T.copy()
            m[(kk < GLOB) | (kk % STRIDE == 0)] = False
            blk.append(dict(kind="B", i=i, k0=k0, keys=kk, m=m))
        for t_ in blk:
            t_["cls"] = ("skip" if not t_["m"].any()
                         else "ones" if t_["m"].all() else "mask")
            if t_["cls"] == "skip":
                t_["qr"] = (0, QB)
            else:
                cols = np.flatnonzero(t_["m"].any(axis=0))
                a, b = int(cols[0]), int(cols[-1]) + 1
                assert (t_["m"].any(axis=0)[a:b]).all()  # contiguous
                t_["qr"] = (a, b)
        # accumulation groups start with tile 0: it must span all q columns
        assert blk[0]["qr"] == (0, QB)
        blocks.append(blk)
    # exactly-once coverage check against the reference mask
    for qb in range(NB):
        cov = np.zeros((QB, T), dtype=np.int32)
        for t_ in blocks[qb]:
            cov[np.arange(QB)[:, None], t_["keys"][None, :]] += t_["m"].T
        assert (cov == full[qb * QB:(qb + 1) * QB].astype(np.int32)).all()
    return blocks


_MASK_TILES = _mask_tiles()


def _build_program():
    nc = bacc.Bacc("TRN2", target_bir_lowering=False, debug=False,
                   enable_asserts=False, num_devices=NCORES)
    bf, f32 = mybir.dt.bfloat16, mybir.dt.float32

    xT = nc.dram_tensor("xT", [D, T], bf, kind="ExternalInput").ap()
    w_q = nc.dram_tensor("w_q", [D, NH * DH], bf, kind="ExternalInput").ap()
    w_dkv = nc.dram_tensor("w_dkv", [D, DL], bf, kind="ExternalInput").ap()
    w_uk = nc.dram_tensor("w_uk", [DL, NH * DH], bf, kind="ExternalInput").ap()
    w_uv = nc.dram_tensor("w_uv", [DL, NH * DH], bf, kind="ExternalInput").ap()
    w_qp = nc.dram_tensor("w_qp", [D, NH * DR], bf, kind="ExternalInput").ap()
    w_kp = nc.dram_tensor("w_kp", [D, NH * DR], bf, kind="ExternalInput").ap()
    w_o = nc.dram_tensor("w_o", [NH * DH, D], bf, kind="ExternalInput").ap()
    cosT = nc.dram_tensor("cosT", [128, T], f32, kind="ExternalInput").ap()
    ssgnT = nc.dram_tensor("ssgnT", [128, T], f32, kind="ExternalInput").ap()
    mA0 = nc.dram_tensor("mA0", [GLOB, QB], bf, kind="ExternalInput").ap()
    mA1 = nc.dram_tensor("mA1", [NB, 32, QB], bf, kind="ExternalInput").ap()
    mB = nc.dram_tensor("mB", [NB, NBW, 128, QB], bf, kind="ExternalInput").ap()
    out = nc.dram_tensor("out", [T, D], bf, kind="ExternalOutput").ap()

    EXP = mybir.ActivationFunctionType.Exp

    with tile.TileContext(nc) as tc:
        with tc.tile_pool(name="acts", bufs=1) as acts, \
             tc.tile_pool(name="consts", bufs=1) as consts:
            q_cT = [acts.tile([128, T], bf, tag=f"q_cT{h}", name=f"q_cT{h}") for h in range(NH)]
            k_cT = [acts.tile([128, T], bf, tag=f"k_cT{h}", name=f"k_cT{h}") for h in range(NH)]
            # per-head q_rope tiles: head h's 64 rope dims live in rows
            # po..po+64 (po = (h%2)*64); the other 64 rows are ZERO so the
            # score matmul can run full-row (128-contraction) against the
            # pair-stacked k_rT stationary -- zeros kill the partner head's
            # contribution.  Full-row MMs keep LDWEIGHTS pull-ahead alive
            # (row_grp MMs were measured to serialize with ~250ns bubbles).
            q_pad = [acts.tile([128, T], bf, tag=f"q_pad{h}", name=f"q_pad{h}") for h in range(NH)]
            k_rT = [acts.tile([128, T], bf, tag=f"k_rT{p}", name=f"k_rT{p}") for p in range(NP)]
            v_t = [acts.tile([128, NH * DH], bf, tag=f"v{t_}", name=f"v{t_}") for t_ in range(NT)]
            v_A = acts.tile([32, NH * DH], bf, tag="v_A", name="v_A")
            # full c_kv latent (4 chunks of 128 dims x T); lives into P2 where
            # the k_c / v matmuls run as work-queue filler between score tiles
            ckv = [acts.tile([128, T], bf, tag=f"ckv{g}", name=f"ckv{g}")
                   for g in range(4)]
            wuk_sb = consts.tile([128, 4 * NH * DH], bf, tag="wuk", name="wuk")
            wuv_sb = consts.tile([128, 4 * NH * DH], bf, tag="wuv", name="wuv")
            ones = consts.tile([128, 128], bf, tag="ones", name="ones")
            nc.vector.memset(ones, 1.0)
            junk = consts.tile([128, 128], bf, tag="junk", name="junk")
            nc.vector.memset(junk, 0.001)
            # zero the pad halves on the (otherwise idle) GpSimd engine so the
            # Vector queue stays clear for the warm-up gating memsets above
            for h in range(NH):
                po = (h % 2) * 64
                nc.gpsimd.memset(q_pad[h][64 - po:128 - po, :], 0.0)
            # load the Exp LUT into ACT early: the first real exp otherwise
            # pays a 1.3us table load right when P2's PSUM rotation is tight
            warm = consts.tile([1, 2], f32, tag="warm", name="warm")
            nc.vector.memset(warm, 0.0)
            nc.scalar.activation(out=warm, in_=warm, func=EXP)

            # ---------------- P1: projections (t-slice streamed) --------------
            with tc.tile_pool(name="wp1", bufs=1) as wp1, \
                 tc.tile_pool(name="xch", bufs=2) as xch, \
                 tc.tile_pool(name="rope_t", bufs=3) as rope_t, \
                 tc.tile_pool(name="p1ps", bufs=7, space="PSUM") as p1ps:
                # warm-up matmuls: no data deps, so they issue from t=0 and
                # keep the PE busy (and the HAM clock un-throttled) while the
                # first x/weight DMAs land
                wu = p1ps.tile([128, SL], f32, tag="proj", name="wu")
                for _ in range(NWARM):
                    nc.tensor.matmul(wu[:, 0:128], ones, junk,
                                     start=True, stop=True)

                # 4-d-tile chunks for w_q and x: Tile's DMA dependency is
                # tile-granular, so chunked tiles let the d=0 matmuls start
                # after ~1/4 of the load.  Chunks are kept coarse (512KB)
                # because every dma_start costs ~650ns of serialized issue
                # time on the Sync engine.
                wq_c = [wp1.tile([128, 4 * NH * DH], bf, tag=f"wq{c}",
                                 name=f"wq{c}") for c in range(4)]
                x0_c = [xch.tile([128, 4 * SL], bf, tag=f"xc{c}",
                                 name=f"xc{c}") for c in range(4)]
                wdkv_sb = wp1.tile([128, 16 * DL], bf, tag="wdkv", name="wdkv")
                wqp_sb = wp1.tile([128, 16 * NH * DR], bf, tag="wqp", name="wqp")
                wkp_sb = wp1.tile([128, 16 * NH * DR], bf, tag="wkp", name="wkp")
                cos_sb = wp1.tile([128, T], f32, tag="cos", name="cos")
                ssg_sb = wp1.tile([128, T], f32, tag="ssg", name="ssg")

                def _wslice(big, cols, d, c0, c1):
                    return big[:, d * cols + c0:d * cols + c1]

                def _load_w(dst, src, cols):
                    nc.sync.dma_start(
                        out=dst.rearrange("p (n m) -> p n m", m=cols),
                        in_=src.rearrange("(n p) m -> p n m", p=128))

                # ordered so slice-0 compute can start ASAP: interleave the
                # x / w_q chunks first (consumed in d order), then the
                # weights in the order the schedule needs them
                wq_r = w_q.rearrange("(n p) m -> p n m", p=128)
                xT_r = xT.rearrange("(n p) m -> p n m", p=128)

                def _load_xc(dst, s, c):
                    nc.sync.dma_start(
                        out=dst.rearrange("p (n m) -> p n m", m=SL),
                        in_=xT_r[:, 4 * c:4 * (c + 1), s * SL:(s + 1) * SL])

                for c in range(4):
                    _load_xc(x0_c[c], 0, c)
                    nc.sync.dma_start(
                        out=wq_c[c].rearrange("p (n m) -> p n m", m=NH * DH),
                        in_=wq_r[:, 4 * c:4 * (c + 1)])
                _load_w(wdkv_sb, w_dkv, DL)
                # slice-1 x right behind wdkv: the scheduler runs slice-1
                # q_c matmuls as soon as data+PSUM allow
                x1_c = [xch.tile([128, 4 * SL], bf, tag=f"xc{c}",
                                 name=f"xc{c}") for c in range(4)]
                for c in range(4):
                    _load_xc(x1_c[c], 1, c)
                _load_w(wqp_sb, w_qp, NH * DR)
                nc.sync.dma_start(out=cos_sb, in_=cosT)
                nc.sync.dma_start(out=ssg_sb, in_=ssgnT)
                _load_w(wkp_sb, w_kp, NH * DR)
                _load_w(wuk_sb, w_uk, NH * DH)
                _load_w(wuv_sb, w_uv, NH * DH)

                xsl = [x0_c, x1_c, None, None]
                for s in range(NS):
                    t0 = s * SL
                    tsl = slice(t0, t0 + SL)
                    if s + 2 < NS:  # prefetch slice s+2 (s+1 already loaded)
                        xsl[s + 2] = [xch.tile([128, 4 * SL], bf, tag=f"xc{c}",
                                               name=f"xc{c}") for c in range(4)]
                        for c in range(4):
                            _load_xc(xsl[s + 2][c], s + 2, c)
                    xcur = xsl[s]
                    xt = [xcur[d // 4][:, (d % 4) * SL:(d % 4 + 1) * SL]
                          for d in range(16)]
                    # content q projections, d-outer so the first matmuls only
                    # need the first DMA chunk of x/w_q (fast start on slice 0)
                    qps = [p1ps.tile([128, SL], f32, tag="proj", name="proj")
                           for _ in range(NH)]
                    for d in range(16):
                        for h in range(NH):
                            nc.tensor.matmul(
                                qps[h],
                                _wslice(wq_c[d // 4], NH * DH, d % 4,
                                        h * DH, (h + 1) * DH),
                                xt[d], start=(d == 0), stop=(d == 15))
                    for h in range(NH):
                        nc.vector.tensor_copy(out=q_cT[h][:, tsl], in_=qps[h])
                    # full c_kv latent for this slice, into the persistent
                    # ckv tiles (4 chunks of 128 latent dims)
                    for g in range(4):
                        ps = p1ps.tile([128, SL], f32, tag="proj", name="proj")
                        for d in range(16):
                            nc.tensor.matmul(
                                ps, _wslice(wdkv_sb, DL, d, g * 128, (g + 1) * 128),
                                xt[d], start=(d == 0), stop=(d == 15))
                        nc.vector.tensor_copy(out=ckv[g][:, tsl], in_=ps)
                    # rope projections + rotation (pair-stacked: 2 heads / tile)
                    for w_sb, isq in ((wqp_sb, True), (wkp_sb, False)):
                        for p in range(NP):
                            ps = p1ps.tile([128, SL], f32, tag="proj", name="proj")
                            for d in range(16):
                                nc.tensor.matmul(
                                    ps, _wslice(w_sb, NH * DR, d, p * 128, (p + 1) * 128),
                                    xt[d], start=(d == 0), stop=(d == 15))
                            m1 = rope_t.tile([128, SL], bf, tag="m1", name="m1")
                            nc.vector.tensor_mul(m1, ps, cos_sb[:, tsl])
                            m2 = rope_t.tile([128, SL], bf, tag="m2", name="m2")
                            for a in (0, 32, 64, 96):
                                sw = a ^ 32
                                nc.vector.tensor_mul(m2[a:a + 32, :],
                                                     ps[sw:sw + 32, :],
                                                     ssg_sb[a:a + 32, tsl])
                            if isq:
                                # split into the per-head zero-padded tiles
                                nc.vector.tensor_add(
                                    q_pad[2 * p][0:64, tsl],
                                    m1[0:64, :], m2[0:64, :])
                                nc.vector.tensor_add(
                                    q_pad[2 * p + 1][64:128, tsl],
                                    m1[64:128, :], m2[64:128, :])
                            else:
                                nc.vector.tensor_add(k_rT[p][:, tsl], m1, m2)

            # ---------------- P2: block-sparse attention ----------------------
            with tc.tile_pool(name="wo", bufs=1) as wo, \
                 tc.tile_pool(name="attp", bufs=1) as attp:
              att = [attp.tile([128, T], bf, tag=f"att{h}", name=f"att{h}")
                     for h in range(NH)]
              w_o_sb = wo.tile([128, NH * D], bf, tag="wo", name="wo")
              nc.sync.dma_start(
                  out=w_o_sb.rearrange("p (n m) -> p n m", m=D),
                  in_=w_o.rearrange("(n p) m -> p n m", p=128))

              with tc.tile_pool(name="mk", bufs=2) as mk, \
                   tc.tile_pool(name="exp", bufs=40) as expp, \
                   tc.tile_pool(name="dacc", bufs=9) as dacc, \
                   tc.tile_pool(name="p2t", bufs=2) as p2t, \
                   tc.tile_pool(name="p3s", bufs=2) as p3s, \
                   tc.tile_pool(name="scps", bufs=5, space="PSUM") as scps, \
                   tc.tile_pool(name="daops", bufs=1, space="PSUM") as daops, \
                   tc.tile_pool(name="p3ps", bufs=2, space="PSUM") as p3ps:
                rcs = {}
                accs = {}
                COPY = mybir.ActivationFunctionType.Copy

                # k_c / v / v_A from the latent, as work-queue chunks popped
                # between score tiles: pure-PE filler that keeps the array
                # busy while the Vector/Scalar engines chew on the exp/acc
                # backlog of the first blocks (previously a serial P1 tail)
                def kv_chunk(kind, s, hu):
                    t0 = s * SL

                    def run():
                        tsl = slice(t0, t0 + SL)
                        if kind == "k":
                            ps = p3ps.tile([128, SL], f32, tag="o", name="kc")
                            for g in range(4):
                                nc.tensor.matmul(
                                    ps,
                                    _wslice(wuk_sb, NH * DH, g,
                                            hu * DH, (hu + 1) * DH),
                                    ckv[g][:, tsl], start=(g == 0),
                                    stop=(g == 3))
                            nc.scalar.activation(out=k_cT[hu][:, tsl], in_=ps,
                                                 func=COPY)
                        elif kind == "v":
                            usl = slice(t0 + hu * 128, t0 + (hu + 1) * 128)
                            ps = p3ps.tile([128, SL], f32, tag="o", name="vc")
                            for g in range(4):
                                nc.tensor.matmul(
                                    ps, ckv[g][:, usl],
                                    wuv_sb[:, g * NH * DH:(g + 1) * NH * DH],
                                    start=(g == 0), stop=(g == 3))
                            nc.vector.tensor_copy(out=v_t[s * 4 + hu], in_=ps)
                        else:  # dilated-key V rows (keys 64j)
                            ps = p3ps.tile([128, SL], f32, tag="o", name="vA")
                            for g in range(4):
                                nc.tensor.matmul(
                                    ps[0:32, :], ckv[g][:, 0:T:STRIDE],
                                    wuv_sb[:, g * NH * DH:(g + 1) * NH * DH],
                                    start=(g == 0), stop=(g == 3))
                            nc.vector.tensor_copy(out=v_A, in_=ps[0:32, :])
                    return run

                def p2_chunks(qb, exps):
                    """pass 2 of block qb as deferred emitters (2 per head).

                    Enqueued as dn0, dn1, ao0, dn2, ao1, dn3, ao2, ao3 so a
                    head's reciprocal (Vector) runs under the next head's dn /
                    interleaved score matmuls instead of stalling the in-order
                    PE queue.
                    """
                    q0 = qb * QB
                    qsl = slice(q0, q0 + QB)
                    blk = [t_ for t_ in _MASK_TILES[qb] if t_["cls"] != "skip"]
                    last = len(blk) - 1

                    def dn_chunk(h):
                        dn = scps.tile([128, QB], f32, tag="sc", name="dn")
                        nc.tensor.matmul(dn, ones, accs.pop((qb, h)),
                                         start=True, stop=True)
                        rc = p2t.tile([128, QB], f32, tag="rc", name="rc")
                        nc.vector.reciprocal_approx_fast(out=rc, in_=dn)
                        rcs[(qb, h)] = rc

                    def ao_chunk(h):
                        hs = slice(h * DH, (h + 1) * DH)
                        ao = daops.tile([128, QB], f32, tag="dao", name="ao")
                        for j, t_ in enumerate(blk):
                            kind, i, k0 = t_["kind"], t_["i"], t_["k0"]
                            rows, ex = exps[(h, kind, i)]
                            if kind == "A0":
                                lv = v_t[0][:, hs]
                            elif kind == "A1":
                                lv = v_A[:, hs]
                            else:
                                lv = v_t[k0 // 128 + i][:, hs]
                            a, b = t_["qr"]
                            nc.tensor.matmul(ao[:, a:b], lv[0:rows, :],
                                             ex[0:rows, 0:b - a],
                                             start=(j == 0), stop=(j == last))
                        nc.vector.tensor_mul(att[h][:, qsl],
                                             ao, rcs.pop((qb, h)))

                    dns = [lambda h=h: dn_chunk(h) for h in range(NH)]
                    aos = [lambda h=h: ao_chunk(h) for h in range(NH)]
                    return [dns[0], dns[1], aos[0], dns[2], aos[1], dns[3],
                            aos[2], aos[3]]

                def p3_chunk(tt):
                    """output projection for one 128-row t-tile."""
                    def run():
                        tsl = slice(128 * tt, 128 * (tt + 1))
                        ob = p3s.tile([128, D], bf, tag="ob", name="ob")
                        for dc in range(4):
                            csl = slice(512 * dc, 512 * (dc + 1))
                            ps = p3ps.tile([128, 512], f32, tag="o", name="o")
                            for h in range(NH):
                                nc.tensor.matmul(ps, att[h][:, tsl],
                                                 w_o_sb[:, h * D + 512 * dc:
                                                        h * D + 512 * (dc + 1)],
                                                 start=(h == 0), stop=(h == 3))
                            # split PSUM->SBUF staging between Vector and the
                            # mostly-idle Scalar engine: a lone Vector gets
                            # cast-bound against the PE here
                            if dc % 2 == 0:
                                nc.vector.tensor_copy(out=ob[:, csl], in_=ps)
                            else:
                                nc.scalar.activation(
                                    out=ob[:, csl], in_=ps,
                                    func=mybir.ActivationFunctionType.Copy)
                        nc.sync.dma_start(out=out[tsl, :], in_=ob)
                    return run

                def _blk_tiles(qb):
                    # A1 depends on the strided full-T k tensors (last P1
                    # slice); emit it last so early score matmuls don't stall.
                    blk = [t_ for t_ in _MASK_TILES[qb] if t_["cls"] != "skip"]
                    return ([t_ for t_ in blk if t_["kind"] != "A1"]
                            + [t_ for t_ in blk if t_["kind"] == "A1"])

                def _load_masks(qb):
                    """kick the mask DMAs for block qb (prefetched one ahead)."""
                    blk = _blk_tiles(qb)
                    msk = {}
                    if any(t_["kind"] == "B" and t_["cls"] == "mask" for t_ in blk):
                        mbs = mk.tile([128, NBW * QB], bf, tag="mbs", name="mbs")
                        nc.sync.dma_start(
                            out=mbs.rearrange("p (n m) -> p n m", m=QB),
                            in_=mB[qb].rearrange("n p m -> p n m"))
                        for t_ in blk:
                            if t_["kind"] == "B" and t_["cls"] == "mask":
                                msk[("B", t_["i"])] = mbs[:, t_["i"] * QB:
                                                          (t_["i"] + 1) * QB]
                    for t_ in blk:
                        if t_["cls"] != "mask" or t_["kind"] == "B":
                            continue
                        rows = t_["m"].shape[0]
                        mt = mk.tile([rows, QB], bf, tag=f"m{t_['kind']}",
                                     name=f"m{t_['kind']}")
                        nc.sync.dma_start(
                            out=mt, in_=mA0 if t_["kind"] == "A0" else mA1[qb])
                        msk[(t_["kind"], t_["i"])] = mt
                    return msk

                work = [("p3", kv_chunk("k", s, h))
                        for s in range(NS) for h in range(NH)]
                work.append(("p3", kv_chunk("A", 0, 0)))
                work.extend(("p3", kv_chunk("v", s, u))
                            for s in range(NS) for u in range(4))
                # pre-drain the k_c chunks the first block's tiles read
                for _ in range(8):
                    work.pop(0)[1]()
                msk_next = _load_masks(0)
                for qb in range(NB):
                    q0 = qb * QB
                    qsl = slice(q0, q0 + QB)
                    blk = _blk_tiles(qb)
                    msk = msk_next
                    if qb + 1 < NB:
                        msk_next = _load_masks(qb + 1)
                    # pass 1: scores + exp + mask, all heads, one key tile at
                    # a time; deferred pass-2/P3 chunks of earlier blocks are
                    # interleaved to keep the PE busy while ACT runs the exps.
                    # Rope matmuls of a head pair are adjacent: disjoint row
                    # groups run concurrently on the PE.
                    exps = {}
                    for jt, t_ in enumerate(blk):
                        npop = (3 if len(work) > 10 else
                                2 if len(work) > 8 else 1)
                        for _ in range(npop):
                            if work:
                                work.pop(0)[1]()
                        kind, i, k0 = t_["kind"], t_["i"], t_["k0"]
                        rows = t_["m"].shape[0]
                        a, b = t_["qr"]
                        w = b - a
                        qv = slice(q0 + a, q0 + b)
                        if kind == "A1":
                            # 32-key dilated tile: pack the 4 heads into
                            # disjoint 32-wide column groups of the PE array —
                            # the 8 matmuls run concurrently on sub-arrays
                            psA = scps.tile([128, QB], f32, tag="sc", name="sc")
                            for h in range(NH):
                                nc.tensor.matmul(
                                    psA[32 * h:32 * h + 32, 0:w],
                                    k_cT[h][:, 0:T:STRIDE], q_cT[h][:, qv],
                                    start=True, stop=False,
                                    tile_position=(0, 32 * h))
                            for h in range(NH):
                                pr, po = h // 2, (h % 2) * 64
                                nc.tensor.matmul(
                                    psA[32 * h:32 * h + 32, 0:w],
                                    k_rT[pr][po:po + 64, 0:T:STRIDE],
                                    q_pad[h][po:po + 64, qv],
                                    start=False, stop=True,
                                    tile_position=(po, 32 * h))
                            pss = [psA[32 * h:32 * h + 32, :] for h in range(NH)]
                        else:
                            # rope matmuls run full-row (128-contraction): the
                            # pair-stacked k_rT is the stationary and q_pad's
                            # zero rows cancel the partner head, so LDWEIGHTS
                            # background pull-ahead keeps the PE back-to-back
                            pss = []
                            for h in range(NH):
                                if kind == "A0":
                                    lk = k_cT[h][:, 0:GLOB]
                                else:
                                    ks = slice(k0 + 128 * i, k0 + 128 * (i + 1))
                                    lk = k_cT[h][:, ks]
                                ps = scps.tile([128, QB], f32, tag="sc", name="sc")
                                nc.tensor.matmul(ps[0:rows, 0:w], lk, q_cT[h][:, qv],
                                                 start=True, stop=False)
                                pss.append(ps)
                            for h in range(NH):
                                pr = h // 2
                                if kind == "A0":
                                    lr = k_rT[pr][:, 0:GLOB]
                                else:
                                    ks = slice(k0 + 128 * i, k0 + 128 * (i + 1))
                                    lr = k_rT[pr][:, ks]
                                nc.tensor.matmul(pss[h][0:rows, 0:w], lr,
                                                 q_pad[h][:, qv],
                                                 start=False, stop=True)
                        for h in range(NH):
                            # elementwise chain stays on Vector: GpSimd was
                            # measured ~8x slower per op (~0.8us fixed cost)
                            # and its latency gates the denominator chain
                            ex = expp.tile([128, QB], bf, tag="ex", name="ex")
                            nc.scalar.activation(out=ex[0:rows, 0:w],
                                                 in_=pss[h][0:rows, 0:w], func=EXP)
                            if t_["cls"] == "mask":
                                nc.vector.tensor_mul(ex[0:rows, 0:w],
                                                     ex[0:rows, 0:w],
                                                     msk[(kind, i)][:, a:b])
                            exps[(h, kind, i)] = (rows, ex)
                            # accumulate the softmax denominator on Vector so
                            # pass 2 needs a single ones-matmul per head
                            if jt == 0:
                                acc = dacc.tile([128, QB], bf, tag="acc",
                                                name="acc")
                                nc.vector.tensor_copy(out=acc, in_=ex)
                                accs[(qb, h)] = acc
                            else:
                                acc = accs[(qb, h)]
                                nc.vector.tensor_add(acc[0:rows, a:b],
                                                     acc[0:rows, a:b],
                                                     ex[0:rows, 0:w])
                    work.extend(("p2", c) for c in p2_chunks(qb, exps))
                    if qb >= 1:
                        work.extend(("p3", p3_chunk(tt)) for tt in
                                    range(4 * (qb - 1), 4 * qb))
                # tail drain: run remaining p3 (pure-PE, inputs ready) chunks
                # between the last block's dn -> reciprocal -> ao chains so
                # the in-order PE queue never stalls on the Vector engine
                p2w = [c for k, c in work if k == "p2"]
                p3w = [c for k, c in work if k == "p3"]
                if p3w:
                    p3w.pop(0)()            # pure PE while Vector flushes the
                                            # last pass-1 exp/acc backlog
                for ch in p2w[:2]:          # dn0, dn1
                    ch()
                if p3w:
                    p3w.pop(0)()            # covers the reciprocals
                for ch in p2w[2:]:          # ao/dn interleaved
                    ch()
                for ch in p3w:
                    ch()
                for tt in range(4 * (NB - 1), 4 * NB):
                    p3_chunk(tt)()
    nc.compile()
    return nc


_NC = None


def _get_nc():
    global _NC
    if _NC is None:
        _NC = _build_program()
    return _NC


def _prep_in_maps(inputs):
    x = np.asarray(inputs["x"], np.float32)
    w_q = np.asarray(inputs["w_q"], np.float32)
    w_dkv = np.asarray(inputs["w_dkv"], np.float32)
    w_uk = np.asarray(inputs["w_uk"], np.float32)
    w_uv = np.asarray(inputs["w_uv"], np.float32)
    w_qp = np.asarray(inputs["w_q_pos"], np.float32)
    w_kp = np.asarray(inputs["w_k_pos"], np.float32)
    w_o = np.asarray(inputs["w_o"], np.float32)

    invf = _inv_freq()                                # [32]
    t = np.arange(T, dtype=np.float32)
    ang = t[None, :] * invf[:, None]                  # [32, T]
    cos32 = np.cos(ang)
    sin32 = np.sin(ang)
    cosT = np.tile(cos32, (4, 1)).astype(np.float32)  # rows p: f = p % 32
    ssgn = np.tile(sin32, (4, 1)).astype(np.float32)
    ssgn[0:32] *= -1.0
    ssgn[64:96] *= -1.0

    mA0 = _MASK_TILES[0][0]["m"].astype(np.float32).astype(BF16)
    mA1 = np.stack([_MASK_TILES[qb][1]["m"] for qb in range(NB)]) \
        .astype(np.float32).astype(BF16)
    mB = np.stack([[_MASK_TILES[qb][2 + i]["m"] for i in range(NBW)]
                   for qb in range(NB)]).astype(np.float32).astype(BF16)

    xT_b = [np.ascontiguousarray(x[b].T).astype(BF16) for b in range(B)]
    common = dict(cosT=cosT, ssgnT=ssgn, mA0=mA0, mA1=mA1, mB=mB,
                  w_dkv=w_dkv.astype(BF16))

    in_maps = []
    for c in range(NCORES):
        b, g = c // 4, c % 4
        ch = slice(4 * g * DH, 4 * (g + 1) * DH)      # content head cols / w_o rows
        rh = slice(4 * g * DR, 4 * (g + 1) * DR)      # rope head cols
        in_maps.append(dict(
            common,
            xT=xT_b[b],
            w_q=(w_q[:, ch] * SCALE).astype(BF16),
            w_uk=np.ascontiguousarray(w_uk[:, ch]).astype(BF16),
            w_uv=np.ascontiguousarray(w_uv[:, ch]).astype(BF16),
            w_qp=(w_qp[:, rh] * (SCALE_ROPE * YARN * YARN)).astype(BF16),
            w_kp=np.ascontiguousarray(w_kp[:, rh]).astype(BF16),
            w_o=np.ascontiguousarray(w_o[ch, :]).astype(BF16),
        ))
    return in_maps


def _run(inputs, trace=False, trace_kwargs=None):
    nc = _get_nc()
    in_maps = _prep_in_maps(inputs)
    res = bass_utils.run_bass_kernel_spmd(
        nc, in_maps, core_ids=list(range(NCORES)), trace=trace,
        **(trace_kwargs or {}))
    out = np.zeros((B, T, D), np.float32)
    for c in range(NCORES):
        out[c // 4] += res.results[c]["out"].astype(np.float32)
    return out, res


def kernel(**inputs) -> np.ndarray:
    out, _ = _run(inputs)
    return out



# revision 30
# speedup vs baseline: 1.0648x; 1.0101x over previous
"""DeepSeek sparse attention (MLA + YaRN RoPE + local/dilated/global mask) on 8 TRN2 cores.

Sharding: (batch, head-group) across 8 cores — core c handles batch c//4, heads
[4*(c%4), 4*(c%4)+4).  Each core computes its projections from the full x (host
pre-transposes x per batch), runs block-sparse attention for its 4 heads, and
produces a row-parallel partial of out @ w_o.  Host sums the 4 partials per batch.

Each core computes the full c_kv latent redundantly: deduplicating it via a
group AllGather was measured to put the chip in a lower power state (~2.0 GHz
PE clock for any collective-enabled NEFF vs 2.4 GHz without), which costs far
more than the duplicated matmuls save.

Layout: "transposed" activations [feature, t] so every matmul keeps the moving
operand in the free dim (N=512/256) at full bf16 rate and no on-chip transposes
are needed anywhere.  Scores are computed as S^T[k, q]; exp tiles are summed on
the Vector engine into one [128, QB] accumulator per (head, block) so the
softmax denominator costs a single ones-matmul; the divide is an elementwise
mul by the reciprocal.
"""

import sys

if "/opt/trn_rl_repo" not in sys.path:
    sys.path.insert(0, "/opt/trn_rl_repo")

import ml_dtypes
import numpy as np

import concourse.bass as bass  # noqa: F401  (bass types used via tile/bacc)
import concourse.mybir as mybir
import concourse.tile as tile
from concourse import bacc, bass_utils

BF16 = ml_dtypes.bfloat16

# ---- problem constants (hardcoded per contract) ----
B, T, D = 2, 2048, 2048
H, DH, DR, DL = 16, 128, 64, 512
WINDOW, STRIDE, GLOB = 512, 64, 128
BASE, MAX_SEQ, ORIG_MAX = 10000.0, 131072, 4096
BETA_FAST, BETA_SLOW = 32.0, 1.0
SCALE = 1.0 / float(np.sqrt(DH))
SCALE_ROPE = 1.0 / float(np.sqrt(DR))
YARN = float(np.float32(0.1 * np.log(MAX_SEQ / ORIG_MAX) + 1.0))
HALF = WINDOW // 2

NCORES = 8
NH = 4            # heads per core
NP = 2            # head-pairs per core (rope tiles stack 2 heads on 128 partitions)
QB = 512          # query block
NB = T // QB      # 4
NBW = (HALF + QB) // 128   # window-strip tiles per block
SL = 512          # t-slice width in projection phase
NS = T // SL      # 4
NT = T // 128     # 16
DLC = DL // 4     # latent dims computed per core (128)
NWARM = 48        # warm-up matmuls: keep PE busy+warm during the input DMA


def _inv_freq():
    base_inv = 1.0 / (BASE ** (np.arange(0, DR, 2, dtype=np.float32) / DR))
    scale = MAX_SEQ / ORIG_MAX
    freqs = np.arange(DR // 2, dtype=np.float32)
    ramp = np.clip((freqs - BETA_SLOW) / (BETA_FAST - BETA_SLOW), 0.0, 1.0)
    return (base_inv * (1 - ramp) + (base_inv / scale) * ramp).astype(np.float32)


def _full_mask():
    pos = np.arange(T)
    qp, kp = pos[:, None], pos[None, :]
    dist = qp - kp
    window = (dist >= -HALF) & (dist <= HALF)
    dil = (kp % STRIDE == 0) | (kp < GLOB)
    return (window | dil) & (kp <= qp)


def _mask_tiles():
    """Per q-block key tiles, with exactly-once ownership masks.

    Tiles: A0 = keys [0, 128) (global), A1 = 32 dilated keys {64j}, B0..B3 =
    the 512-wide sliding window strip.  A0 owns k<128; A1 owns k%64==0 & k>=128;
    B owns the rest.  Each tile classified: 'skip' (all-zero), 'ones', 'mask'.
    """
    full = _full_mask()
    blocks = []
    for qb in range(NB):
        q0 = qb * QB
        k0 = max(0, q0 - HALF)
        qs = slice(q0, q0 + QB)
        blk = []
        m = full[qs, 0:GLOB].T.copy()                        # [128, QB]
        blk.append(dict(kind="A0", i=0, k0=0, keys=np.arange(GLOB), m=m))
        keys = np.arange(32) * STRIDE
        m = full[qs, :][:, keys].T.copy()                    # [32, QB]
        m[keys < GLOB] = False
        blk.append(dict(kind="A1", i=0, k0=0, keys=keys, m=m))
        for i in range(NBW):
            kk = k0 + 128 * i + np.arange(128)
            m = full[qs, :][:, kk].T.copy()
            m[(kk < GLOB) | (kk % STRIDE == 0)] = False
            blk.append(dict(kind="B", i=i, k0=k0, keys=kk, m=m))
        for t_ in blk:
            t_["cls"] = ("skip" if not t_["m"].any()
                         else "ones" if t_["m"].all() else "mask")
            if t_["cls"] == "skip":
                t_["qr"] = (0, QB)
            else:
                cols = np.flatnonzero(t_["m"].any(axis=0))
                a, b = int(cols[0]), int(cols[-1]) + 1
                assert (t_["m"].any(axis=0)[a:b]).all()  # contiguous
                t_["qr"] = (a, b)
        # accumulation groups start with tile 0: it must span all q columns
        assert blk[0]["qr"] == (0, QB)
        blocks.append(blk)
    # exactly-once coverage check against the reference mask
    for qb in range(NB):
        cov = np.zeros((QB, T), dtype=np.int32)
        for t_ in blocks[qb]:
            cov[np.arange(QB)[:, None], t_["keys"][None, :]] += t_["m"].T
        assert (cov == full[qb * QB:(qb + 1) * QB].astype(np.int32)).all()
    return blocks


_MASK_TILES = _mask_tiles()


def _build_program():
    nc = bacc.Bacc("TRN2", target_bir_lowering=False, debug=False,
                   enable_asserts=False, num_devices=NCORES)
    bf, f32 = mybir.dt.bfloat16, mybir.dt.float32

    xT = nc.dram_tensor("xT", [D, T], bf, kind="ExternalInput").ap()
    w_q = nc.dram_tensor("w_q", [D, NH * DH], bf, kind="ExternalInput").ap()
    w_dkv = nc.dram_tensor("w_dkv", [D, DL], bf, kind="ExternalInput").ap()
    w_uk = nc.dram_tensor("w_uk", [DL, NH * DH], bf, kind="ExternalInput").ap()
    w_uv = nc.dram_tensor("w_uv", [DL, NH * DH], bf, kind="ExternalInput").ap()
    w_qp = nc.dram_tensor("w_qp", [D, NH * DR], bf, kind="ExternalInput").ap()
    w_kp = nc.dram_tensor("w_kp", [D, NH * DR], bf, kind="ExternalInput").ap()
    w_o = nc.dram_tensor("w_o", [NH * DH, D], bf, kind="ExternalInput").ap()
    cosT = nc.dram_tensor("cosT", [128, T], f32, kind="ExternalInput").ap()
    ssgnT = nc.dram_tensor("ssgnT", [128, T], f32, kind="ExternalInput").ap()
    mA0 = nc.dram_tensor("mA0", [GLOB, QB], bf, kind="ExternalInput").ap()
    mA1 = nc.dram_tensor("mA1", [NB, 32, QB], bf, kind="ExternalInput").ap()
    mB = nc.dram_tensor("mB", [NB, NBW, 128, QB], bf, kind="ExternalInput").ap()
    out = nc.dram_tensor("out", [T, D], bf, kind="ExternalOutput").ap()

    EXP = mybir.ActivationFunctionType.Exp

    with tile.TileContext(nc) as tc:
        with tc.tile_pool(name="acts", bufs=1) as acts, \
             tc.tile_pool(name="consts", bufs=1) as consts:
            q_cT = [acts.tile([128, T], bf, tag=f"q_cT{h}", name=f"q_cT{h}") for h in range(NH)]
            k_cT = [acts.tile([128, T], bf, tag=f"k_cT{h}", name=f"k_cT{h}") for h in range(NH)]
            # per-head q_rope tiles: head h's 64 rope dims live in rows
            # po..po+64 (po = (h%2)*64); the other 64 rows are ZERO so the
            # score matmul can run full-row (128-contraction) against the
            # pair-stacked k_rT stationary -- zeros kill the partner head's
            # contribution.  Full-row MMs keep LDWEIGHTS pull-ahead alive
            # (row_grp MMs were measured to serialize with ~250ns bubbles).
            q_pad = [acts.tile([128, T], bf, tag=f"q_pad{h}", name=f"q_pad{h}") for h in range(NH)]
            k_rT = [acts.tile([128, T], bf, tag=f"k_rT{p}", name=f"k_rT{p}") for p in range(NP)]
            v_t = [acts.tile([128, NH * DH], bf, tag=f"v{t_}", name=f"v{t_}") for t_ in range(NT)]
            v_A = acts.tile([32, NH * DH], bf, tag="v_A", name="v_A")
            # full c_kv latent (4 chunks of 128 dims x T); lives into P2 where
            # the k_c / v matmuls run as work-queue filler between score tiles
            ckv = [acts.tile([128, T], bf, tag=f"ckv{g}", name=f"ckv{g}")
                   for g in range(4)]
            wuk_sb = consts.tile([128, 4 * NH * DH], bf, tag="wuk", name="wuk")
            wuv_sb = consts.tile([128, 4 * NH * DH], bf, tag="wuv", name="wuv")
            ones = consts.tile([128, 128], bf, tag="ones", name="ones")
            nc.vector.memset(ones, 1.0)
            junk = consts.tile([128, 128], bf, tag="junk", name="junk")
            nc.vector.memset(junk, 0.001)
            # zero the pad halves on the (otherwise idle) GpSimd engine so the
            # Vector queue stays clear for the warm-up gating memsets above
            for h in range(NH):
                po = (h % 2) * 64
                nc.gpsimd.memset(q_pad[h][64 - po:128 - po, :], 0.0)
            # load the Exp LUT into ACT early: the first real exp otherwise
            # pays a 1.3us table load right when P2's PSUM rotation is tight
            warm = consts.tile([1, 2], f32, tag="warm", name="warm")
            nc.vector.memset(warm, 0.0)
            nc.scalar.activation(out=warm, in_=warm, func=EXP)

            # ---------------- P1: projections (t-slice streamed) --------------
            with tc.tile_pool(name="wp1", bufs=1) as wp1, \
                 tc.tile_pool(name="xch", bufs=2) as xch, \
                 tc.tile_pool(name="rope_t", bufs=3) as rope_t, \
                 tc.tile_pool(name="p1ps", bufs=7, space="PSUM") as p1ps:
                # warm-up matmuls: no data deps, so they issue from t=0 and
                # keep the PE busy (and the HAM clock un-throttled) while the
                # first x/weight DMAs land
                wu = p1ps.tile([128, SL], f32, tag="proj", name="wu")
                for _ in range(NWARM):
                    nc.tensor.matmul(wu[:, 0:128], ones, junk,
                                     start=True, stop=True)

                # 4-d-tile chunks for w_q and x: Tile's DMA dependency is
                # tile-granular, so chunked tiles let the d=0 matmuls start
                # after ~1/4 of the load.  Chunks are kept coarse (512KB)
                # because every dma_start costs ~650ns of serialized issue
                # time on the Sync engine.
                wq_c = [wp1.tile([128, 4 * NH * DH], bf, tag=f"wq{c}",
                                 name=f"wq{c}") for c in range(4)]
                x0_c = [xch.tile([128, 4 * SL], bf, tag=f"xc{c}",
                                 name=f"xc{c}") for c in range(4)]
                wdkv_sb = wp1.tile([128, 16 * DL], bf, tag="wdkv", name="wdkv")
                wqp_sb = wp1.tile([128, 16 * NH * DR], bf, tag="wqp", name="wqp")
                wkp_sb = wp1.tile([128, 16 * NH * DR], bf, tag="wkp", name="wkp")
                cos_sb = wp1.tile([128, T], f32, tag="cos", name="cos")
                ssg_sb = wp1.tile([128, T], f32, tag="ssg", name="ssg")

                def _wslice(big, cols, d, c0, c1):
                    return big[:, d * cols + c0:d * cols + c1]

                def _load_w(dst, src, cols):
                    nc.sync.dma_start(
                        out=dst.rearrange("p (n m) -> p n m", m=cols),
                        in_=src.rearrange("(n p) m -> p n m", p=128))

                # ordered so slice-0 compute can start ASAP: interleave the
                # x / w_q chunks first (consumed in d order), then the
                # weights in the order the schedule needs them
                wq_r = w_q.rearrange("(n p) m -> p n m", p=128)
                xT_r = xT.rearrange("(n p) m -> p n m", p=128)

                def _load_xc(dst, s, c):
                    nc.sync.dma_start(
                        out=dst.rearrange("p (n m) -> p n m", m=SL),
                        in_=xT_r[:, 4 * c:4 * (c + 1), s * SL:(s + 1) * SL])

                for c in range(4):
                    _load_xc(x0_c[c], 0, c)
                    nc.sync.dma_start(
                        out=wq_c[c].rearrange("p (n m) -> p n m", m=NH * DH),
                        in_=wq_r[:, 4 * c:4 * (c + 1)])
                # wdkv split in two and interleaved with the slice-1 x
                # chunks, matching the d-ordered consumption of the c_kv and
                # slice-1 q_c matmul chains (the first ~30us is DMA-bound)
                wdkv_r2 = wdkv_sb.rearrange("p (n m) -> p n m", m=DL)
                wdkv_src = w_dkv.rearrange("(n p) m -> p n m", p=128)
                x1_c = [xch.tile([128, 4 * SL], bf, tag=f"xc{c}",
                                 name=f"xc{c}") for c in range(4)]
                nc.sync.dma_start(out=wdkv_r2[:, 0:8], in_=wdkv_src[:, 0:8])
                _load_xc(x1_c[0], 1, 0)
                _load_xc(x1_c[1], 1, 1)
                nc.sync.dma_start(out=wdkv_r2[:, 8:16], in_=wdkv_src[:, 8:16])
                _load_xc(x1_c[2], 1, 2)
                _load_xc(x1_c[3], 1, 3)
                _load_w(wqp_sb, w_qp, NH * DR)
                nc.sync.dma_start(out=cos_sb, in_=cosT)
                nc.sync.dma_start(out=ssg_sb, in_=ssgnT)
                _load_w(wkp_sb, w_kp, NH * DR)
                _load_w(wuk_sb, w_uk, NH * DH)
                _load_w(wuv_sb, w_uv, NH * DH)

                xsl = [x0_c, x1_c, None, None]
                for s in range(NS):
                    t0 = s * SL
                    tsl = slice(t0, t0 + SL)
                    if s + 2 < NS:  # prefetch slice s+2 (s+1 already loaded)
                        xsl[s + 2] = [xch.tile([128, 4 * SL], bf, tag=f"xc{c}",
                                               name=f"xc{c}") for c in range(4)]
                        for c in range(4):
                            _load_xc(xsl[s + 2][c], s + 2, c)
                    xcur = xsl[s]
                    xt = [xcur[d // 4][:, (d % 4) * SL:(d % 4 + 1) * SL]
                          for d in range(16)]
                    # content q projections, d-outer so the first matmuls only
                    # need the first DMA chunk of x/w_q (fast start on slice 0)
                    qps = [p1ps.tile([128, SL], f32, tag="proj", name="proj")
                           for _ in range(NH)]
                    for d in range(16):
                        for h in range(NH):
                            nc.tensor.matmul(
                                qps[h],
                                _wslice(wq_c[d // 4], NH * DH, d % 4,
                                        h * DH, (h + 1) * DH),
                                xt[d], start=(d == 0), stop=(d == 15))
                    for h in range(NH):
                        nc.vector.tensor_copy(out=q_cT[h][:, tsl], in_=qps[h])
                    # full c_kv latent for this slice, into the persistent
                    # ckv tiles (4 chunks of 128 latent dims)
                    for g in range(4):
                        ps = p1ps.tile([128, SL], f32, tag="proj", name="proj")
                        for d in range(16):
                            nc.tensor.matmul(
                                ps, _wslice(wdkv_sb, DL, d, g * 128, (g + 1) * 128),
                                xt[d], start=(d == 0), stop=(d == 15))
                        nc.vector.tensor_copy(out=ckv[g][:, tsl], in_=ps)
                    # rope projections + rotation (pair-stacked: 2 heads / tile)
                    for w_sb, isq in ((wqp_sb, True), (wkp_sb, False)):
                        for p in range(NP):
                            ps = p1ps.tile([128, SL], f32, tag="proj", name="proj")
                            for d in range(16):
                                nc.tensor.matmul(
                                    ps, _wslice(w_sb, NH * DR, d, p * 128, (p + 1) * 128),
                                    xt[d], start=(d == 0), stop=(d == 15))
                            m1 = rope_t.tile([128, SL], bf, tag="m1", name="m1")
                            nc.vector.tensor_mul(m1, ps, cos_sb[:, tsl])
                            m2 = rope_t.tile([128, SL], bf, tag="m2", name="m2")
                            for a in (0, 32, 64, 96):
                                sw = a ^ 32
                                nc.vector.tensor_mul(m2[a:a + 32, :],
                                                     ps[sw:sw + 32, :],
                                                     ssg_sb[a:a + 32, tsl])
                            if isq:
                                # split into the per-head zero-padded tiles
                                nc.vector.tensor_add(
                                    q_pad[2 * p][0:64, tsl],
                                    m1[0:64, :], m2[0:64, :])
                                nc.vector.tensor_add(
                                    q_pad[2 * p + 1][64:128, tsl],
                                    m1[64:128, :], m2[64:128, :])
                            else:
                                nc.vector.tensor_add(k_rT[p][:, tsl], m1, m2)

            # ---------------- P2: block-sparse attention ----------------------
            with tc.tile_pool(name="wo", bufs=1) as wo, \
                 tc.tile_pool(name="attp", bufs=1) as attp:
              att = [attp.tile([128, T], bf, tag=f"att{h}", name=f"att{h}")
                     for h in range(NH)]
              w_o_sb = wo.tile([128, NH * D], bf, tag="wo", name="wo")
              nc.sync.dma_start(
                  out=w_o_sb.rearrange("p (n m) -> p n m", m=D),
                  in_=w_o.rearrange("(n p) m -> p n m", p=128))

              with tc.tile_pool(name="mk", bufs=2) as mk, \
                   tc.tile_pool(name="exp", bufs=40) as expp, \
                   tc.tile_pool(name="dacc", bufs=9) as dacc, \
                   tc.tile_pool(name="p2t", bufs=2) as p2t, \
                   tc.tile_pool(name="p3s", bufs=2) as p3s, \
                   tc.tile_pool(name="scps", bufs=5, space="PSUM") as scps, \
                   tc.tile_pool(name="daops", bufs=1, space="PSUM") as daops, \
                   tc.tile_pool(name="p3ps", bufs=2, space="PSUM") as p3ps:
                rcs = {}
                accs = {}
                COPY = mybir.ActivationFunctionType.Copy

                # k_c / v / v_A from the latent, as work-queue chunks popped
                # between score tiles: pure-PE filler that keeps the array
                # busy while the Vector/Scalar engines chew on the exp/acc
                # backlog of the first blocks (previously a serial P1 tail)
                def kv_chunk(kind, s, hu):
                    t0 = s * SL

                    def run():
                        tsl = slice(t0, t0 + SL)
                        if kind == "k":
                            ps = p3ps.tile([128, SL], f32, tag="o", name="kc")
                            for g in range(4):
                                nc.tensor.matmul(
                                    ps,
                                    _wslice(wuk_sb, NH * DH, g,
                                            hu * DH, (hu + 1) * DH),
                                    ckv[g][:, tsl], start=(g == 0),
                                    stop=(g == 3))
                            nc.scalar.activation(out=k_cT[hu][:, tsl], in_=ps,
                                                 func=COPY)
                        elif kind == "v":
                            usl = slice(t0 + hu * 128, t0 + (hu + 1) * 128)
                            ps = p3ps.tile([128, SL], f32, tag="o", name="vc")
                            for g in range(4):
                                nc.tensor.matmul(
                                    ps, ckv[g][:, usl],
                                    wuv_sb[:, g * NH * DH:(g + 1) * NH * DH],
                                    start=(g == 0), stop=(g == 3))
                            nc.vector.tensor_copy(out=v_t[s * 4 + hu], in_=ps)
                        else:  # dilated-key V rows (keys 64j)
                            ps = p3ps.tile([128, SL], f32, tag="o", name="vA")
                            for g in range(4):
                                nc.tensor.matmul(
                                    ps[0:32, :], ckv[g][:, 0:T:STRIDE],
                                    wuv_sb[:, g * NH * DH:(g + 1) * NH * DH],
                                    start=(g == 0), stop=(g == 3))
                            nc.vector.tensor_copy(out=v_A, in_=ps[0:32, :])
                    return run

                def p2_chunks(qb, exps):
                    """pass 2 of block qb as deferred emitters (2 per head).

                    Enqueued as dn0, dn1, ao0, dn2, ao1, dn3, ao2, ao3 so a
                    head's reciprocal (Vector) runs under the next head's dn /
                    interleaved score matmuls instead of stalling the in-order
                    PE queue.
                    """
                    q0 = qb * QB
                    qsl = slice(q0, q0 + QB)
                    blk = [t_ for t_ in _MASK_TILES[qb] if t_["cls"] != "skip"]
                    last = len(blk) - 1

                    def dn_chunk(h):
                        dn = scps.tile([128, QB], f32, tag="sc", name="dn")
                        nc.tensor.matmul(dn, ones, accs.pop((qb, h)),
                                         start=True, stop=True)
                        rc = p2t.tile([128, QB], f32, tag="rc", name="rc")
                        nc.vector.reciprocal_approx_fast(out=rc, in_=dn)
                        rcs[(qb, h)] = rc

                    def ao_chunk(h):
                        hs = slice(h * DH, (h + 1) * DH)
                        ao = daops.tile([128, QB], f32, tag="dao", name="ao")
                        for j, t_ in enumerate(blk):
                            kind, i, k0 = t_["kind"], t_["i"], t_["k0"]
                            rows, ex = exps[(h, kind, i)]
                            if kind == "A0":
                                lv = v_t[0][:, hs]
                            elif kind == "A1":
                                lv = v_A[:, hs]
                            else:
                                lv = v_t[k0 // 128 + i][:, hs]
                            a, b = t_["qr"]
                            nc.tensor.matmul(ao[:, a:b], lv[0:rows, :],
                                             ex[0:rows, 0:b - a],
                                             start=(j == 0), stop=(j == last))
                        nc.vector.tensor_mul(att[h][:, qsl],
                                             ao, rcs.pop((qb, h)))

                    dns = [lambda h=h: dn_chunk(h) for h in range(NH)]
                    aos = [lambda h=h: ao_chunk(h) for h in range(NH)]
                    return [dns[0], dns[1], aos[0], dns[2], aos[1], dns[3],
                            aos[2], aos[3]]

                def p3_chunk(tt):
                    """output projection for one 128-row t-tile."""
                    def run():
                        tsl = slice(128 * tt, 128 * (tt + 1))
                        ob = p3s.tile([128, D], bf, tag="ob", name="ob")
                        for dc in range(4):
                            csl = slice(512 * dc, 512 * (dc + 1))
                            ps = p3ps.tile([128, 512], f32, tag="o", name="o")
                            for h in range(NH):
                                nc.tensor.matmul(ps, att[h][:, tsl],
                                                 w_o_sb[:, h * D + 512 * dc:
                                                        h * D + 512 * (dc + 1)],
                                                 start=(h == 0), stop=(h == 3))
                            # split PSUM->SBUF staging between Vector and the
                            # mostly-idle Scalar engine: a lone Vector gets
                            # cast-bound against the PE here
                            if dc % 2 == 0:
                                nc.vector.tensor_copy(out=ob[:, csl], in_=ps)
                            else:
                                nc.scalar.activation(
                                    out=ob[:, csl], in_=ps,
                                    func=mybir.ActivationFunctionType.Copy)
                        nc.sync.dma_start(out=out[tsl, :], in_=ob)
                    return run

                def _blk_tiles(qb):
                    # A1 depends on the strided full-T k tensors (last P1
                    # slice); emit it last so early score matmuls don't stall.
                    blk = [t_ for t_ in _MASK_TILES[qb] if t_["cls"] != "skip"]
                    return ([t_ for t_ in blk if t_["kind"] != "A1"]
                            + [t_ for t_ in blk if t_["kind"] == "A1"])

                def _load_masks(qb):
                    """kick the mask DMAs for block qb (prefetched one ahead)."""
                    blk = _blk_tiles(qb)
                    msk = {}
                    if any(t_["kind"] == "B" and t_["cls"] == "mask" for t_ in blk):
                        mbs = mk.tile([128, NBW * QB], bf, tag="mbs", name="mbs")
                        nc.sync.dma_start(
                            out=mbs.rearrange("p (n m) -> p n m", m=QB),
                            in_=mB[qb].rearrange("n p m -> p n m"))
                        for t_ in blk:
                            if t_["kind"] == "B" and t_["cls"] == "mask":
                                msk[("B", t_["i"])] = mbs[:, t_["i"] * QB:
                                                          (t_["i"] + 1) * QB]
                    for t_ in blk:
                        if t_["cls"] != "mask" or t_["kind"] == "B":
                            continue
                        rows = t_["m"].shape[0]
                        mt = mk.tile([rows, QB], bf, tag=f"m{t_['kind']}",
                                     name=f"m{t_['kind']}")
                        nc.sync.dma_start(
                            out=mt, in_=mA0 if t_["kind"] == "A0" else mA1[qb])
                        msk[(t_["kind"], t_["i"])] = mt
                    return msk

                work = [("p3", kv_chunk("k", s, h))
                        for s in range(NS) for h in range(NH)]
                work.append(("p3", kv_chunk("A", 0, 0)))
                work.extend(("p3", kv_chunk("v", s, u))
                            for s in range(NS) for u in range(4))
                # pre-drain the k_c chunks the first block's tiles read
                for _ in range(8):
                    work.pop(0)[1]()
                msk_next = _load_masks(0)
                for qb in range(NB):
                    q0 = qb * QB
                    qsl = slice(q0, q0 + QB)
                    blk = _blk_tiles(qb)
                    msk = msk_next
                    if qb + 1 < NB:
                        msk_next = _load_masks(qb + 1)
                    # pass 1: scores + exp + mask, all heads, one key tile at
                    # a time; deferred pass-2/P3 chunks of earlier blocks are
                    # interleaved to keep the PE busy while ACT runs the exps.
                    # Rope matmuls of a head pair are adjacent: disjoint row
                    # groups run concurrently on the PE.
                    exps = {}
                    for jt, t_ in enumerate(blk):
                        # p2 chunks (dn/ao) gate on the previous block's
                        # Vector acc backlog -- popping one early wedges the
                        # in-order PE queue, so hold them for the first two
                        # tiles; kv/p3 filler is ungated and pops freely.
                        # (Deeper lag deadlocks: the exp pool (40) must fit
                        # the previous block's 32 live tiles + 4 per lagged
                        # tile before the ao pops release them.)
                        for _ in range(3):
                            if work and (work[0][0] != "p2" or jt >= 2):
                                work.pop(0)[1]()
                        kind, i, k0 = t_["kind"], t_["i"], t_["k0"]
                        rows = t_["m"].shape[0]
                        a, b = t_["qr"]
                        w = b - a
                        qv = slice(q0 + a, q0 + b)
                        if kind == "A1":
                            # 32-key dilated tile: pack the 4 heads into
                            # disjoint 32-wide column groups of the PE array —
                            # the 8 matmuls run concurrently on sub-arrays
                            psA = scps.tile([128, QB], f32, tag="sc", name="sc")
                            for h in range(NH):
                                nc.tensor.matmul(
                                    psA[32 * h:32 * h + 32, 0:w],
                                    k_cT[h][:, 0:T:STRIDE], q_cT[h][:, qv],
                                    start=True, stop=False,
                                    tile_position=(0, 32 * h))
                            for h in range(NH):
                                pr, po = h // 2, (h % 2) * 64
                                nc.tensor.matmul(
                                    psA[32 * h:32 * h + 32, 0:w],
                                    k_rT[pr][po:po + 64, 0:T:STRIDE],
                                    q_pad[h][po:po + 64, qv],
                                    start=False, stop=True,
                                    tile_position=(po, 32 * h))
                            pss = [psA[32 * h:32 * h + 32, :] for h in range(NH)]
                        else:
                            # rope matmuls run full-row (128-contraction): the
                            # pair-stacked k_rT is the stationary and q_pad's
                            # zero rows cancel the partner head, so LDWEIGHTS
                            # background pull-ahead keeps the PE back-to-back
                            pss = []
                            for h in range(NH):
                                if kind == "A0":
                                    lk = k_cT[h][:, 0:GLOB]
                                else:
                                    ks = slice(k0 + 128 * i, k0 + 128 * (i + 1))
                                    lk = k_cT[h][:, ks]
                                ps = scps.tile([128, QB], f32, tag="sc", name="sc")
                                nc.tensor.matmul(ps[0:rows, 0:w], lk, q_cT[h][:, qv],
                                                 start=True, stop=False)
                                pss.append(ps)
                            for h in range(NH):
                                pr = h // 2
                                if kind == "A0":
                                    lr = k_rT[pr][:, 0:GLOB]
                                else:
                                    ks = slice(k0 + 128 * i, k0 + 128 * (i + 1))
                                    lr = k_rT[pr][:, ks]
                                nc.tensor.matmul(pss[h][0:rows, 0:w], lr,
                                                 q_pad[h][:, qv],
                                                 start=False, stop=True)
                        for h in range(NH):
                            # elementwise chain stays on Vector: GpSimd was
                            # measured ~8x slower per op (~0.8us fixed cost)
                            # and its latency gates the denominator chain
                            ex = expp.tile([128, QB], bf, tag="ex", name="ex")
                            nc.scalar.activation(out=ex[0:rows, 0:w],
                                                 in_=pss[h][0:rows, 0:w], func=EXP)
                            if t_["cls"] == "mask":
                                nc.vector.tensor_mul(ex[0:rows, 0:w],
                                                     ex[0:rows, 0:w],
                                                     msk[(kind, i)][:, a:b])
                            exps[(h, kind, i)] = (rows, ex)
                            # accumulate the softmax denominator on Vector so
                            # pass 2 needs a single ones-matmul per head
                            if jt == 0:
                                acc = dacc.tile([128, QB], bf, tag="acc",
                                                name="acc")
                                nc.vector.tensor_copy(out=acc, in_=ex)
                                accs[(qb, h)] = acc
                            else:
                                acc = accs[(qb, h)]
                                nc.vector.tensor_add(acc[0:rows, a:b],
                                                     acc[0:rows, a:b],
                                                     ex[0:rows, 0:w])
                    work.extend(("p2", c) for c in p2_chunks(qb, exps))
                    if qb >= 1:
                        work.extend(("p3", p3_chunk(tt)) for tt in
                                    range(4 * (qb - 1), 4 * qb))
                # tail drain: run remaining p3 (pure-PE, inputs ready) chunks
                # between the last block's dn -> reciprocal -> ao chains so
                # the in-order PE queue never stalls on the Vector engine
                p2w = [c for k, c in work if k == "p2"]
                p3w = [c for k, c in work if k == "p3"]
                if p3w:
                    p3w.pop(0)()            # pure PE while Vector flushes the
                                            # last pass-1 exp/acc backlog
                for ch in p2w[:2]:          # dn0, dn1
                    ch()
                if p3w:
                    p3w.pop(0)()            # covers the reciprocals
                for ch in p2w[2:]:          # ao/dn interleaved
                    ch()
                for ch in p3w:
                    ch()
                for tt in range(4 * (NB - 1), 4 * NB):
                    p3_chunk(tt)()
    nc.compile()
    return nc


_NC = None


def _get_nc():
    global _NC
    if _NC is None:
        _NC = _build_program()
    return _NC


def _prep_in_maps(inputs):
    x = np.asarray(inputs["x"], np.float32)
    w_q = np.asarray(inputs["w_q"], np.float32)
    w_dkv = np.asarray(inputs["w_dkv"], np.float32)
    w_uk = np.asarray(inputs["w_uk"], np.float32)
    w_uv = np.asarray(inputs["w_uv"], np.float32)
    w_qp = np.asarray(inputs["w_q_pos"], np.float32)
    w_kp = np.asarray(inputs["w_k_pos"], np.float32)
    w_o = np.asarray(inputs["w_o"], np.float32)

    invf = _inv_freq()                                # [32]
    t = np.arange(T, dtype=np.float32)
    ang = t[None, :] * invf[:, None]                  # [32, T]
    cos32 = np.cos(ang)
    sin32 = np.sin(ang)
    cosT = np.tile(cos32, (4, 1)).astype(np.float32)  # rows p: f = p % 32
    ssgn = np.tile(sin32, (4, 1)).astype(np.float32)
    ssgn[0:32] *= -1.0
    ssgn[64:96] *= -1.0

    mA0 = _MASK_TILES[0][0]["m"].astype(np.float32).astype(BF16)
    mA1 = np.stack([_MASK_TILES[qb][1]["m"] for qb in range(NB)]) \
        .astype(np.float32).astype(BF16)
    mB = np.stack([[_MASK_TILES[qb][2 + i]["m"] for i in range(NBW)]
                   for qb in range(NB)]).astype(np.float32).astype(BF16)

    xT_b = [np.ascontiguousarray(x[b].T).astype(BF16) for b in range(B)]
    common = dict(cosT=cosT, ssgnT=ssgn, mA0=mA0, mA1=mA1, mB=mB,
                  w_dkv=w_dkv.astype(BF16))

    in_maps = []
    for c in range(NCORES):
        b, g = c // 4, c % 4
        ch = slice(4 * g * DH, 4 * (g + 1) * DH)      # content head cols / w_o rows
        rh = slice(4 * g * DR, 4 * (g + 1) * DR)      # rope head cols
        in_maps.append(dict(
            common,
            xT=xT_b[b],
            w_q=(w_q[:, ch] * SCALE).astype(BF16),
            w_uk=np.ascontiguousarray(w_uk[:, ch]).astype(BF16),
            w_uv=np.ascontiguousarray(w_uv[:, ch]).astype(BF16),
            w_qp=(w_qp[:, rh] * (SCALE_ROPE * YARN * YARN)).astype(BF16),
            w_kp=np.ascontiguousarray(w_kp[:, rh]).astype(BF16),
            w_o=np.ascontiguousarray(w_o[ch, :]).astype(BF16),
        ))
    return in_maps


def _run(inputs, trace=False, trace_kwargs=None):
    nc = _get_nc()
    in_maps = _prep_in_maps(inputs)
    res = bass_utils.run_bass_kernel_spmd(
        nc, in_maps, core_ids=list(range(NCORES)), trace=trace,
        **(trace_kwargs or {}))
    out = np.zeros((B, T, D), np.float32)
    for c in range(NCORES):
        out[c // 4] += res.results[c]["out"].astype(np.float32)
    return out, res


def kernel(**inputs) -> np.ndarray:
    out, _ = _run(inputs)
    return out

